# revision 15
# baseline (speedup 1.0000x reference)
"""Trainium2 8-core kernel for nn_Attention_70892730187933 (sparse multi-query attention).

Sharding: sequence-parallel over query rows. Core c owns rows {i : i % 8 == c},
as 2 blocks of 128 rows (block0 < 1024, block1 >= 1024). Key space padded to
17*128 = 2176 (incl. 2 null cols). No collectives; host concatenates rows.

v2 design vs baseline:
- Causal q-column trimming: for key tile jt, only q columns [S_jt, 256) per
  head can attend (S_jt = max(0, 16*jt-1), worst-case over cores); per-pair
  sim width drops 6656 -> 4384. q is packed head-major so each tile is one
  contiguous per-head slice.
- Bias is applied multiplicatively: host packs exp(bias) (0 where masked) and
  DVE multiplies it into exp(sim) at bf16 2x rate. This removes the
  identity-matmul bias adds (half of all sim PE work) entirely.
- Softmax normalization is inline: rowsums ride as a ones-column in V; DVE
  takes reciprocals straight from the PSUM rowsum row (no DRAM roundtrips)
  and per-head normalization overlaps the pair loop.
- DMA issue order prioritizes the critical path: consts, xq/xtq (LN chain),
  wkv + xt (kv chain, split in 4 column slices gating kv matmul chunks), wq,
  then bias tiles / wout.

Raw Block + explicit semaphores (this walrus build rejects multi-wait
instructions); the planner records semaphore counter targets at plan time,
then emits all four engine programs inside one Block.
"""

import sys
import numpy as np

sys.path.insert(0, "/opt/trn_rl_repo")

B, N, DIM, HEADS, DIM_HEAD, NUM_NULL = 1, 2048, 1024, 16, 64, 2
INNER = HEADS * DIM_HEAD
EPS = 1e-5
NCORES = 8
JT = 17
JPAD = JT * 128
NQ = 256
MASK_VAL = -30000.0
NPAIR = HEADS // 2

# per-head q-col start for key tile jt (worst case over cores => widest)
S_JT = [max(0, 16 * jt - 1) for jt in range(JT)]
W_JT = [256 - s for s in S_JT]          # per-head width
TW = [2 * w for w in W_JT]              # per-pair tile width (h0|h1 packed)
# E/bias pair layout: [h0 tiles packed (EW1) | h1 tiles packed (EW1)]
ECOL1 = [0] * JT
for _jt in range(1, JT):
    ECOL1[_jt] = ECOL1[_jt - 1] + W_JT[_jt - 1]
EW1 = ECOL1[-1] + W_JT[-1]              # 2192 per-head packed width
EW = 2 * EW1                            # 4384 packed pair E/bias width

# sim psum chunks: tiles packed into regions A(<=1536), B(<=1536), A(<=1536)
# region layout per chunk: [h0 tiles (L/2) | h1 tiles (L/2)]
CHUNKS = [[0, 1, 2], [3, 4, 5, 6], list(range(7, JT))]
CH_LEN = [sum(TW[j] for j in ck) for ck in CHUNKS]    # 1444, 1480, 1460
CH_E1 = [ECOL1[ck[0]] for ck in CHUNKS]               # per-head col offset
NCK = len(CHUNKS)

_CACHE = {}


def _build_graph():
    from contextlib import ExitStack
    import concourse.bass as bass
    import concourse.mybir as mybir

    dt = mybir.dt
    F32, BF16 = dt.float32, dt.bfloat16
    AF = mybir.ActivationFunctionType
    OP = mybir.AluOpType
    AX = mybir.AxisListType
    nc = bass.Bass()

    # all DRAM parameters are host-prearranged to match their SBUF layout
    # exactly (contiguous [128, W] rows -> 128 large DMA descriptors each);
    # xt is chunk-slice-major: [p, chs, ct, 512] so each of the 4 column
    # slices is one contiguous row-chunk
    # bcon packs bf16 consts+weights: ibf(0:128) ones(128:256) srow(256:1280)
    # nkvt(1280:1282) wkv(1282:2306); xf packs f32: xrow(0:2048)
    # xtq(2048:4096) if32(4096:4224) onesf(4224:4352)
    d_in = {}
    for name, shape, ty in [
        ("bcon", [128, 2306], BF16), ("xf", [128, 4352], F32),
        ("xt", [128, 8 * N], BF16), ("wq", [128, 8 * INNER], BF16),
        ("wout", [64, 16 * DIM], BF16), ("biasp", [NPAIR, 128, EW], BF16),
    ]:
        d_in[name] = nc.declare_dram_parameter(name, shape, ty, isOutput=False)
    out_d = nc.declare_dram_parameter("out", [DIM, NQ], F32, isOutput=True)

    ctx = ExitStack()
    sb = {}
    for name, shape, ty in [
        ("bcon", [128, 2306], BF16), ("xf", [128, 4352], F32),
        ("zb", [128, 1], F32), ("epsb", [128, 1], F32),
        ("wq", [128, 8 * INNER], BF16), ("wout", [64, 16 * DIM], BF16),
        ("xt", [128, 8 * N], BF16),
        ("xc", [128, DIM], F32),
        ("lns", [128, 12], F32),
        ("rsq_row", [1, NQ], F32), ("nmr_row", [1, NQ], F32),
        ("rsqb", [128, NQ], F32), ("negmurs", [1, NQ], BF16),
        ("xst", [128, 8 * NQ], BF16), ("qtmp", [128, 2 * NQ], BF16),
        ("kv", [128, JPAD], BF16), ("vsb", [128, JT * 65], BF16),
        ("e0", [128, EW], BF16), ("e1", [128, EW], BF16),
        ("b0", [128, EW], BF16), ("b1", [128, EW], BF16),
        ("eraw0", [128, CH_LEN[0]], BF16), ("eraw1", [128, CH_LEN[1]], BF16),
        ("eraw2", [128, CH_LEN[2]], BF16),
        ("oT", [64, HEADS * NQ], BF16),
        ("rrow", [1, 512], F32), ("recipflat", [1, HEADS * NQ], BF16),
        ("oTn", [64, 8 * NQ], BF16), ("oTn_lo", [64, 8 * NQ], BF16),
        ("outsb", [128, 3 * NQ], F32),
    ] + [(f"qh{h}", [64, 2 * NQ], BF16) for h in range(NPAIR)]:
        sb[name] = ctx.enter_context(nc.sbuf_tensor("sb_" + name, shape, ty))
    bc, xfm = sb["bcon"], sb["xf"]
    BC_IBF, BC_ONE, BC_SROW, BC_NKV, BC_WKV = 0, 128, 256, 1280, 1282
    XF_XR, XF_XTQ, XF_I32, XF_ONE = 0, 2048, 4096, 4224

    qh = [sb[f"qh{h}"] for h in range(NPAIR)]
    esb = [sb["e0"], sb["e1"]]
    bsb = [sb["b0"], sb["b1"]]
    eraw = [sb["eraw0"], sb["eraw1"], sb["eraw2"]]

    # PSUM: early tensors freed before pair-loop tensors are allocated.
    early = ExitStack()
    kvp = [early.enter_context(nc.psum_tensor(f"kvp{i}", [128, 512], F32))
           for i in range(2)]
    qp = [early.enter_context(nc.psum_tensor(f"qp{i}", [128, NQ], F32))
          for i in range(2)]
    vp = [early.enter_context(nc.psum_tensor(f"vp{i}", [128, 64], BF16))
          for i in range(2)]
    stp = early.enter_context(nc.psum_tensor("stp", [1, 128], F32))
    rbp = early.enter_context(nc.psum_tensor("rbp", [128, NQ], F32))
    early.close()
    simA = ctx.enter_context(nc.psum_tensor("simA", [128, 1536], F32))
    simB = ctx.enter_context(nc.psum_tensor("simB", [128, 1536], F32))
    opp = ctx.enter_context(nc.psum_tensor("opp", [65, 512], F32))
    nrm = ctx.enter_context(nc.psum_tensor("nrm", [64, 512], F32))
    SIMREG = [simA, simB]

    # ------- planner -------
    plan = {"sync": [], "tensor": [], "vector": [], "scalar": []}
    DSEMS = (("dbc", "dxf", "dwq", "dw", "db0", "db1", "do0", "do1", "do2")
             + tuple(f"dk{i}" for i in range(4))
             + tuple(f"dq{i}" for i in range(8)))
    cnt = {"p": 0, "v": 0, "s": 0, **{k: 0 for k in DSEMS}}
    SEM = {}

    def wait(eng, sem, thr):
        if thr > 0:
            plan[eng].append(lambda e, s=sem, t=thr: e.wait_ge(SEM[s], t))

    def dma(sem, out, in_):
        cnt[sem] += 16
        plan["sync"].append(
            lambda e, s=sem, o=out, i=in_: e.dma_start(out=o, in_=i)
            .then_inc(SEM[s], 16))
        return cnt[sem]

    def inc(eng, sem, fn):
        cnt[sem] += 1
        if eng in ("vector", "scalar"):
            plan[eng].append(lambda e, f=fn: f(e))
            plan[eng].append(lambda e, s=sem: e.drain().then_inc(SEM[s], 1))
        else:
            plan[eng].append(lambda e, f=fn, s=sem: f(e).then_inc(SEM[s], 1))
        return cnt[sem]

    def run(eng, fn):
        plan[eng].append(fn)
        if eng in ("vector", "scalar"):
            plan[eng].append(lambda e: e.drain())

    # ========== SYNC: initial loads in priority order ==========
    d_bc = dma("dbc", bc[:], d_in["bcon"][:])
    d_xf = dma("dxf", xfm[:], d_in["xf"][:])
    # xt split into 4 contiguous slices so kv matmul chunk ch gates on slice ch
    for chs in range(4):
        dma(f"dk{chs}", sb["xt"][:, chs * 4096:(chs + 1) * 4096],
            d_in["xt"][:, chs * 4096:(chs + 1) * 4096])
    d_wq = dma("dwq", sb["wq"][:], d_in["wq"][:])
    for p in range(2):
        dma(f"db{p}", bsb[p][:], d_in["biasp"][p])
    d_w = dma("dw", sb["wout"][:], d_in["wout"][:])

    # ========== VECTOR: memsets ==========
    run("vector", lambda e: e.memset(sb["zb"][:], 0.0))
    run("vector", lambda e: e.memset(sb["epsb"][:], EPS))
    run("vector", lambda e: e.memset(sb["vsb"][:], 1.0))
    run("vector", lambda e: e.memset(sb["kv"][:, NUM_NULL + N:JPAD], 0.0))
    wait("vector", "dbc", d_bc)
    v_memset = inc("vector", "v", lambda e: e.tensor_copy(
        sb["kv"][:, 0:NUM_NULL], bc[:, BC_NKV:BC_NKV + NUM_NULL]))

    # ========== LN stats: lns cols t*6 + {0 negmu, 1 ssq, 2 lnv, 3 rsqc, 4 nmrc}
    v_center = [0, 0]
    s_sq = [0, 0]
    s_rsqc = [0, 0]
    v_nmrc = [0, 0]
    for t in range(2):
        c0 = t * 6
        negmu = sb["lns"][:, c0:c0 + 1]
        if t == 0:
            wait("vector", "dxf", d_xf)
        if t == 1:
            wait("vector", "s", s_sq[0])  # xc reuse
        run("vector", lambda e, t=t, negmu=negmu: e.tensor_reduce(
            out=negmu, in_=xfm[:, t * DIM:(t + 1) * DIM],
            axis=AX.X, op=OP.add, negate=True))
        run("vector", lambda e, negmu=negmu: e.tensor_scalar_mul(
            out=negmu, in0=negmu, scalar1=1.0 / DIM))
        v_center[t] = inc("vector", "v", lambda e, t=t, negmu=negmu:
                          e.tensor_scalar_add(
                              out=sb["xc"][:],
                              in0=xfm[:, t * DIM:(t + 1) * DIM],
                              scalar1=negmu))
        # scalar chain for this t
        if t == 0:
            wait("scalar", "v", v_memset)
        wait("scalar", "v", v_center[t])
        ssq = sb["lns"][:, c0 + 1:c0 + 2]
        lnv = sb["lns"][:, c0 + 2:c0 + 3]
        rsqc = sb["lns"][:, c0 + 3:c0 + 4]
        s_sq[t] = inc("scalar", "s", lambda e, t=t, ssq=ssq: e.activation(
            out=xfm[:, t * DIM:(t + 1) * DIM], in_=sb["xc"][:],
            func=AF.Square, bias=sb["zb"][:], accum_out=ssq))
        run("scalar", lambda e, ssq=ssq, lnv=lnv: e.activation(
            out=lnv, in_=ssq, func=AF.Ln, scale=1.0 / DIM, bias=sb["epsb"][:]))
        s_rsqc[t] = inc("scalar", "s", lambda e, lnv=lnv, rsqc=rsqc: e.activation(
            out=rsqc, in_=lnv, func=AF.Exp, scale=-0.5, bias=sb["zb"][:]))
        wait("vector", "s", s_rsqc[t])
        v_nmrc[t] = inc("vector", "v", lambda e, c0=c0: e.tensor_tensor(
            out=sb["lns"][:, c0 + 4:c0 + 5], in0=sb["lns"][:, c0:c0 + 1],
            in1=sb["lns"][:, c0 + 3:c0 + 4], op=OP.mult))

    # ========== TENSOR: kv matmuls (kvp double-buffered) ==========
    p_kvchunk = [0] * 4
    s_kvevac = [0] * 4
    for ch in range(4):
        pb = kvp[ch % 2]
        if ch == 0:
            wait("tensor", "dbc", d_bc)
        wait("tensor", f"dk{ch}", 16)
        if ch >= 2:
            wait("tensor", "s", s_kvevac[ch - 2])
        for ct in range(8):
            fn = lambda e, pb=pb, ch=ch, ct=ct: e.matmul(
                pb[:], bc[:, BC_WKV + ct * 128:BC_WKV + (ct + 1) * 128],
                sb["xt"][:, ch * 4096 + ct * 512:ch * 4096 + (ct + 1) * 512],
                start=(ct == 0), stop=(ct == 7))
            if ct == 7:
                p_kvchunk[ch] = inc("tensor", "p", fn)
            else:
                run("tensor", fn)
        wait("scalar", "p", p_kvchunk[ch])
        s_kvevac[ch] = inc("scalar", "s", lambda e, pb=pb, ch=ch: e.activation(
            out=sb["kv"][:, NUM_NULL + ch * 512:NUM_NULL + (ch + 1) * 512],
            in_=pb[:], func=AF.Copy))

    # ========== TENSOR: stats transposes + rsqb broadcast ==========
    v_statrow = [[0, 0], [0, 0]]
    wait("tensor", "dxf", d_xf)   # if32/onesf loaded
    for t in range(2):
        c0 = t * 6
        wait("tensor", "s", s_rsqc[t])
        if t == 1:
            wait("tensor", "v", v_statrow[0][1])  # stp reuse
        pst = inc("tensor", "p", lambda e, c0=c0: e.transpose(
            stp[:], sb["lns"][:, c0 + 3:c0 + 4], xfm[:, XF_I32:XF_I32 + 128]))
        wait("vector", "p", pst)
        v_statrow[t][0] = inc("vector", "v", lambda e, t=t: e.tensor_copy(
            sb["rsq_row"][0:1, t * 128:(t + 1) * 128], stp[:]))
        wait("tensor", "v", v_statrow[t][0])
        wait("tensor", "v", v_nmrc[t])
        pst2 = inc("tensor", "p", lambda e, c0=c0: e.transpose(
            stp[:], sb["lns"][:, c0 + 4:c0 + 5], xfm[:, XF_I32:XF_I32 + 128]))
        wait("vector", "p", pst2)
        v_statrow[t][1] = inc("vector", "v", lambda e, t=t: e.tensor_copy(
            sb["nmr_row"][0:1, t * 128:(t + 1) * 128], stp[:]))

    wait("tensor", "v", v_statrow[1][0])
    p_rsqb = inc("tensor", "p", lambda e: e.matmul(
        rbp[:], xfm[0:1, XF_ONE:XF_ONE + 128], sb["rsq_row"][0:1, :],
        start=True, stop=True))
    wait("vector", "p", p_rsqb)
    run("vector", lambda e: e.tensor_copy(sb["rsqb"][:], rbp[:]))
    v_negmurs = inc("vector", "v",
                    lambda e: e.tensor_copy(sb["negmurs"][:], sb["nmr_row"][0:1, :]))
    for ct in range(8):
        fn = lambda e, ct=ct: e.tensor_tensor(
            out=sb["xst"][:, ct * NQ:(ct + 1) * NQ],
            in0=xfm[:, XF_XTQ + ct * NQ:XF_XTQ + (ct + 1) * NQ],
            in1=sb["rsqb"][:], op=OP.mult)
        if ct == 7:
            v_xst = inc("vector", "v", fn)
        else:
            run("vector", fn)

    # ========== TENSOR: v transposes (vp double-buffered) ==========
    p_vt = [0] * JT
    s_vcopy = [0] * JT
    for jt in range(JT):
        pb = vp[jt % 2]
        ch_hi = min(3, ((jt + 1) * 128 - 1 - NUM_NULL) // 512)
        wait("tensor", "s", s_kvevac[ch_hi])
        if jt == 0:
            wait("tensor", "v", v_memset)
        if jt >= 2:
            wait("tensor", "s", s_vcopy[jt - 2])
        p_vt[jt] = inc("tensor", "p", lambda e, pb=pb, jt=jt: e.transpose(
            pb[:], sb["kv"][64:128, jt * 128:(jt + 1) * 128],
            bc[64:128, BC_IBF + 64:BC_IBF + 128]))
        wait("scalar", "p", p_vt[jt])
        s_vcopy[jt] = inc("scalar", "s", lambda e, pb=pb, jt=jt: e.activation(
            out=sb["vsb"][:, jt * 65:jt * 65 + 64], in_=pb[:], func=AF.Copy))
    s_vsb = s_vcopy[JT - 1]

    # ========== TENSOR: q projection (qp double-buffered), head-major evac ===
    wait("tensor", "v", v_xst)
    wait("tensor", "dwq", d_wq)
    wait("tensor", "dbc", d_bc)
    p_q = [0] * 8
    v_qtmp = [0] * 8
    for dtile in range(8):
        pb = qp[dtile % 2]
        if dtile >= 2:
            wait("tensor", "v", v_qtmp[dtile - 2])
        for ct in range(8):
            run("tensor", lambda e, pb=pb, dtile=dtile, ct=ct: e.matmul(
                pb[:],
                sb["wq"][:, ct * INNER + dtile * 128:ct * INNER + (dtile + 1) * 128],
                sb["xst"][:, ct * NQ:(ct + 1) * NQ],
                start=(ct == 0), stop=False))
        p_q[dtile] = inc("tensor", "p", lambda e, pb=pb, dtile=dtile: e.matmul(
            pb[:], bc[0:1, BC_SROW + dtile * 128:BC_SROW + (dtile + 1) * 128],
            sb["negmurs"][:], start=False, stop=True))
        wait("vector", "p", p_q[dtile])
        # even head (psum rows 0:64) -> qh[p][:, 0:256] directly
        run("vector", lambda e, pb=pb, dtile=dtile: e.tensor_copy(
            qh[dtile][0:64, 0:NQ], pb[0:64, :]))
        slot = dtile % 2
        if dtile >= 2:
            wait("vector", f"dq{dtile - 2}", 16)  # qtmp slot reuse
        v_qtmp[dtile] = inc("vector", "v", lambda e, pb=pb, slot=slot:
                            e.tensor_copy(
                                sb["qtmp"][64:128, slot * NQ:(slot + 1) * NQ],
                                pb[64:128, :]))
        wait("sync", "v", v_qtmp[dtile])
        dma(f"dq{dtile}", qh[dtile][0:64, NQ:2 * NQ],
            sb["qtmp"][64:128, slot * NQ:(slot + 1) * NQ])

    # ========== PAIR LOOP ==========
    v_pre = v_qtmp[7]
    p_simc = [[0] * NCK for _ in range(NPAIR)]
    s_exp = [[0] * NCK for _ in range(NPAIR)]
    v_mult = [[0] * NCK for _ in range(NPAIR)]
    p_odone = [0] * NPAIR
    v_oevac = [0] * NPAIR
    p_bcast = [0] * NPAIR
    v_normmult = [0] * NPAIR

    def emit_fill(p, ci):
        # regions alternate by global chunk index: reuse guard is the exp of
        # the chunk two slots earlier, which finished two chunk-periods ago
        g = NCK * p + ci
        ps = SIMREG[g % 2]
        if g >= 2:
            pp, cp = divmod(g - 2, NCK)
            wait("tensor", "s", s_exp[pp][cp])
        if p == 0 and ci == 0:
            wait("tensor", "v", v_pre)      # early psum drained (qp/rbp/stp)
            wait("tensor", "s", s_vsb)      # vp drained + kvp via kvevacs
        if ci == 0:
            wait("tensor", f"dq{p}", 16)
        ch_hi = min(3, ((CHUNKS[ci][-1] + 1) * 128 - 1 - NUM_NULL) // 512)
        if p == 0:
            wait("tensor", "s", s_kvevac[ch_hi])
        base = CH_E1[ci]
        half = CH_LEN[ci] // 2
        # build emission list of bank-safe pieces, then set start on the
        # first piece touching each psum bank and stop on the last (start
        # zeroes the whole 2KB bank; one start/stop pair per bank per group)
        pieces = []
        for jt in CHUNKS[ci]:
            s, w = S_JT[jt], W_JT[jt]
            for h in range(2):
                a0 = h * half + (ECOL1[jt] - base)
                a, b = a0, a0 + w
                while a < b:
                    cut = min(b, (a // 512 + 1) * 512)
                    qa = h * 256 + s + (a - a0)
                    pieces.append([jt, a, cut, qa, qa + (cut - a)])
                    a = cut
        first_in_bank = {}
        last_in_bank = {}
        for pi, (jt, a, b, qa, qb) in enumerate(pieces):
            first_in_bank.setdefault(a // 512, pi)
            last_in_bank[a // 512] = pi
        for pi, (jt, a, b, qa, qb) in enumerate(pieces):
            st = first_in_bank[a // 512] == pi
            sp = last_in_bank[a // 512] == pi
            fn = lambda e, ps=ps, jt=jt, a=a, b=b, qa=qa, qb=qb, st=st, \
                sp=sp: e.matmul(
                    ps[:, a:b], sb["kv"][0:64, jt * 128:(jt + 1) * 128],
                    qh[p][0:64, qa:qb], start=st, stop=sp)
            if pi == len(pieces) - 1:
                p_simc[p][ci] = inc("tensor", "p", fn)
            else:
                run("tensor", fn)

    def emit_ogroup(p):
        eh_ = esb[p % 2]
        if p == 0:
            wait("tensor", "s", s_vsb)
        if p >= 1:
            wait("tensor", "v", v_oevac[p - 1])   # opp reuse
        for ci in range(NCK):
            wait("tensor", "v", v_mult[p][ci])
            for jt in CHUNKS[ci]:
                s, w = S_JT[jt], W_JT[jt]
                for h in range(2):
                    fn = lambda e, jt=jt, s=s, w=w, h=h, eh_=eh_: e.matmul(
                        opp[0:65, h * 256 + s:(h + 1) * 256],
                        sb["vsb"][:, jt * 65:jt * 65 + 65],
                        eh_[:, h * EW1 + ECOL1[jt]:h * EW1 + ECOL1[jt] + w],
                        start=(jt == 0 and h == 0),
                        stop=(jt == JT - 1 and h == 1))
                    if jt == JT - 1 and h == 1:
                        p_odone[p] = inc("tensor", "p", fn)
                    else:
                        run("tensor", fn)

    def emit_bcast(p):
        wait("tensor", "v", v_oevac[p])      # recipflat(p) ready
        if p >= 1:
            wait("tensor", "v", v_normmult[p - 1])   # nrm reuse
        p_bcast[p] = inc("tensor", "p", lambda e, p=p: e.matmul(
            nrm[:], bc[0:1, BC_ONE:BC_ONE + 64],
            sb["recipflat"][0:1, p * 512:(p + 1) * 512], start=True, stop=True))

    def emit_mults(p, ci):
        eh_ = esb[p % 2]
        bh_ = bsb[p % 2]
        wait("vector", "s", s_exp[p][ci])
        if ci == 0:
            wait("vector", f"db{p % 2}", 16 * (p // 2 + 1))
            if p >= 2:
                wait("vector", "p", p_odone[p - 2])   # eh slot reuse
        base = CH_E1[ci]
        half = CH_LEN[ci] // 2
        run("vector", lambda e, half=half, base=base, eh_=eh_, bh_=bh_,
            ci=ci: e.tensor_tensor(
                out=eh_[:, base:base + half], in0=eraw[ci][:, 0:half],
                in1=bh_[:, base:base + half], op=OP.mult))
        v_mult[p][ci] = inc("vector", "v", lambda e, half=half, base=base,
                            eh_=eh_, bh_=bh_, ci=ci: e.tensor_tensor(
            out=eh_[:, EW1 + base:EW1 + base + half],
            in0=eraw[ci][:, half:2 * half],
            in1=bh_[:, EW1 + base:EW1 + base + half], op=OP.mult))

    def emit_oevac(p):
        wait("vector", "p", p_odone[p])
        run("vector", lambda e, p=p: e.tensor_copy(
            sb["oT"][0:64, p * 512:(p + 1) * 512], opp[0:64, :]))
        run("vector", lambda e: e.reciprocal(
            out=sb["rrow"][0:1, :], in_=opp[64:65, :]))
        v_oevac[p] = inc("vector", "v", lambda e, p=p: e.tensor_copy(
            sb["recipflat"][0:1, p * 512:(p + 1) * 512], sb["rrow"][0:1, :]))

    def emit_normmult(p):
        wait("vector", "p", p_bcast[p])
        run("vector", lambda e, p=p: e.tensor_tensor(
            out=sb["oTn"][:, p * NQ:(p + 1) * NQ], in0=nrm[0:64, 0:256],
            in1=sb["oT"][0:64, p * 512:p * 512 + 256], op=OP.mult))
        v_normmult[p] = inc("vector", "v", lambda e, p=p: e.tensor_tensor(
            out=sb["oTn_lo"][0:64, p * NQ:(p + 1) * NQ], in0=nrm[0:64, 256:512],
            in1=sb["oT"][0:64, p * 512 + 256:(p + 1) * 512], op=OP.mult))

    def emit_exp(p, ci):
        wait("scalar", "p", p_simc[p][ci])
        if p >= 1:
            wait("scalar", "v", v_mult[p - 1][ci])    # eraw slot reuse
        ps = SIMREG[(NCK * p + ci) % 2]
        ln = CH_LEN[ci]
        s_exp[p][ci] = inc("scalar", "s", lambda e, ps=ps, ln=ln, ci=ci:
                           e.activation(out=eraw[ci][:, 0:ln], in_=ps[:, 0:ln],
                                        func=AF.Exp, bias=sb["zb"][:]))

    for p in range(NPAIR):
        # call order matters only for plan-time counter availability;
        # engine programs are built per-engine in the order emitted below
        emit_fill(p, 0)
        emit_fill(p, 1)
        emit_exp(p, 0)
        emit_exp(p, 1)
        if p >= 1:
            emit_ogroup(p - 1)
        if p >= 2:
            emit_bcast(p - 2)
        if p >= 1:
            emit_oevac(p - 1)
        if p >= 2:
            emit_normmult(p - 2)
        emit_mults(p, 0)
        emit_fill(p, 2)
        emit_exp(p, 2)
        emit_mults(p, 1)
        emit_mults(p, 2)
        # SYNC: bias prefetch for pair p+2
        if p + 2 < NPAIR:
            wait("sync", "v", v_mult[p][2])
            dma(f"db{(p + 2) % 2}", bsb[(p + 2) % 2][:], d_in["biasp"][p + 2])

    # epilogue
    emit_ogroup(NPAIR - 1)
    emit_bcast(NPAIR - 2)
    emit_oevac(NPAIR - 1)
    emit_normmult(NPAIR - 2)
    emit_bcast(NPAIR - 1)
    emit_normmult(NPAIR - 1)

    # ========== output projection (accumulators alternate in simA) ==========
    # per head-half matmuls read oTn (even heads) / oTn_lo (odd) directly;
    # per-hdt waits let early heads accumulate while late pairs normalize
    wait("tensor", "dw", d_w)
    wait("tensor", "s", s_exp[NPAIR - 1][2])   # sim regions free
    s_outevac = [0] * 8
    OREGS = [simA[:, 0:NQ], simA[:, 512:512 + NQ], simA[:, 1024:1024 + NQ]]
    for et in range(8):
        reg = OREGS[et % 3]
        if et >= 3:
            wait("tensor", "s", s_outevac[et - 3])
        p_wout = 0
        for hdt in range(8):
            if et == 0:
                wait("tensor", "v", v_normmult[hdt])
            for hf in range(2):
                mv = sb["oTn"] if hf == 0 else sb["oTn_lo"]
                c0 = hf * 8 * DIM + hdt * DIM + et * 128
                fn = lambda e, c0=c0, mv=mv, reg=reg, hdt=hdt, hf=hf: e.matmul(
                    reg, sb["wout"][0:64, c0:c0 + 128],
                    mv[:, hdt * NQ:(hdt + 1) * NQ],
                    start=(hdt == 0 and hf == 0), stop=(hdt == 7 and hf == 1))
                if hdt == 7 and hf == 1:
                    p_wout = inc("tensor", "p", fn)
                else:
                    run("tensor", fn)
        wait("scalar", "p", p_wout)
        oslot = et % 3
        if et >= 3:
            wait("scalar", f"do{oslot}", 16 * (et // 3))  # outsb slot reuse
        s_outevac[et] = inc("scalar", "s", lambda e, oslot=oslot, reg=reg:
                            e.activation(
                                out=sb["outsb"][:, oslot * NQ:(oslot + 1) * NQ],
                                in_=reg, func=AF.Copy))
        wait("sync", "s", s_outevac[et])
        dma(f"do{oslot}", out_d[et * 128:(et + 1) * 128, :],
            sb["outsb"][:, oslot * NQ:(oslot + 1) * NQ])

    # ========== emit ==========
    from contextlib import ExitStack as _ES
    semctx = _ES()
    for k in ("p", "v", "s") + DSEMS:
        SEM[k] = semctx.enter_context(nc.semaphore(f"sem_{k}"))
    with semctx:
        with nc.Block() as block:
            @block.sync
            def _(e):
                for fn in plan["sync"]:
                    fn(e)

            @block.tensor
            def _(e):
                for fn in plan["tensor"]:
                    fn(e)

            @block.vector
            def _(e):
                for fn in plan["vector"]:
                    fn(e)

            @block.scalar
            def _(e):
                for fn in plan["scalar"]:
                    fn(e)
    ctx.close()
    return nc


def _prep_inputs(x, attn_bias, Wq, Wkv, null_kv, Wout, gamma, mask):
    from ml_dtypes import bfloat16
    x = np.asarray(x, np.float32)[0]            # [N, DIM]
    attn_bias = np.asarray(attn_bias, np.float32)[0]  # [H, N, N]
    Wq = np.asarray(Wq, np.float32)
    Wkv = np.asarray(Wkv, np.float32)
    null_kv = np.asarray(null_kv, np.float32)
    Wout = np.asarray(Wout, np.float32)
    gamma = np.asarray(gamma, np.float32)
    mask = np.asarray(mask, bool)[0]            # [N]

    scale = DIM_HEAD ** -0.5
    wq_eff = (gamma[:, None] * Wq * scale).astype(np.float32)
    srow = wq_eff.sum(axis=0, keepdims=True)
    xt = np.ascontiguousarray(x.T)
    nkvt = np.zeros((128, NUM_NULL), np.float32)
    nkvt[0:DIM_HEAD, :] = null_kv[0].T
    nkvt[64:64 + DIM_HEAD, :] = null_kv[1].T
    I128 = np.eye(128, dtype=np.float32)
    ones = np.ones((1, 128), np.float32)

    jpad = np.arange(JPAD)
    jvalid = np.zeros(JPAD, bool)
    jvalid[:NUM_NULL] = True
    jvalid[NUM_NULL:NUM_NULL + N] = mask
    key_of_j = jpad - NUM_NULL

    in_maps = []
    idx_all = []
    for c in range(NCORES):
        idx = np.concatenate([np.arange(c, 1024, 8), np.arange(1024 + c, 2048, 8)])
        idx_all.append(idx)
        allow = jvalid[None, :] & (key_of_j[None, :] <= idx[:, None])  # [NQ, JPAD]
        allow[:, :NUM_NULL] = True
        ab = np.zeros((HEADS, JPAD, NQ), np.float32)
        ab[:, NUM_NULL:NUM_NULL + N, :] = attn_bias[:, idx, :].transpose(0, 2, 1)
        bt = np.where(allow.T[None], ab, MASK_VAL)
        ebt = np.exp(bt)                 # exp(bias); exactly 0 where masked
        # pack per head-PAIR, trimmed per-tile: [h0 (w) | h1 (w)] at ECOL[jt]
        pk = np.empty((NPAIR, 128, EW), np.float32)
        for jt in range(JT):
            s, w = S_JT[jt], W_JT[jt]
            c0 = ECOL1[jt]
            tile = ebt[:, jt * 128:(jt + 1) * 128, s:256]     # [H, 128, w]
            pk[:, :, c0:c0 + w] = tile[0::2]
            pk[:, :, EW1 + c0:EW1 + c0 + w] = tile[1::2]
        bcon = np.zeros((128, 2306), np.float32)
        bcon[:, 0:128] = I128
        bcon[0, 128:256] = 1.0
        bcon[0, 256:1280] = srow[0]
        bcon[:, 1280:1282] = nkvt
        bcon[:, 1282:2306] = Wkv.reshape(8, 128, 128).transpose(1, 0, 2) \
            .reshape(128, -1)
        xf = np.zeros((128, 4352), np.float32)
        xf[:, 0:2048] = x[idx].reshape(2, 128, DIM).transpose(1, 0, 2) \
            .reshape(128, -1)
        xf[:, 2048:4096] = xt[:, idx].reshape(8, 128, NQ).transpose(1, 0, 2) \
            .reshape(128, -1)
        xf[:, 4096:4224] = I128
        xf[0, 4224:4352] = 1.0
        in_maps.append({
            "bcon": bcon.astype(bfloat16),
            "xf": xf,
            "xt": np.ascontiguousarray(
                xt.reshape(8, 128, 4, 512).transpose(1, 2, 0, 3)
                .reshape(128, -1)).astype(bfloat16),
            "wq": np.ascontiguousarray(
                wq_eff.reshape(8, 128, INNER).transpose(1, 0, 2)
                .reshape(128, -1)).astype(bfloat16),
            "wout": np.ascontiguousarray(
                Wout.reshape(8, 2, 64, DIM).transpose(1, 2, 0, 3)
                .reshape(2, 64, -1).transpose(1, 0, 2)
                .reshape(64, -1)).astype(bfloat16),
            "biasp": pk.astype(bfloat16),
        })
    return in_maps, idx_all


def _run(inputs, trace=False):
    from concourse.bass_utils import run_bass_kernel_spmd
    if "nc" not in _CACHE:
        _CACHE["nc"] = _build_graph()
    nc = _CACHE["nc"]
    in_maps, idx_all = _prep_inputs(**inputs)
    res = run_bass_kernel_spmd(nc, in_maps, list(range(NCORES)), trace=trace)
    out = np.zeros((B, N, DIM), np.float32)
    for c in range(NCORES):
        out[0, idx_all[c], :] = res.results[c]["out"].T
    return out, res


def kernel(**inputs):
    out, _ = _run(inputs, trace=False)
    return out


# revision 16
# speedup vs baseline: 1.3085x; 1.3085x over previous
"""Trainium2 8-core kernel for nn_Attention_70892730187933 (sparse multi-query attention).

Sharding: sequence-parallel over query rows. Core c owns rows {i : i % 8 == c},
as 2 blocks of 128 rows (block0 < 1024, block1 >= 1024). Key space padded to
17*128 = 2176 (incl. 2 null cols). No collectives; host concatenates rows.

v2 design vs baseline:
- Causal q-column trimming: for key tile jt, only q columns [S_jt, 256) per
  head can attend (S_jt = max(0, 16*jt-1), worst-case over cores); per-pair
  sim width drops 6656 -> 4384. q is packed head-major so each tile is one
  contiguous per-head slice.
- Bias is applied multiplicatively: host packs exp(bias) (0 where masked) and
  DVE multiplies it into exp(sim) at bf16 2x rate. This removes the
  identity-matmul bias adds (half of all sim PE work) entirely.
- Softmax normalization is inline: rowsums ride as a ones-column in V; DVE
  takes reciprocals straight from the PSUM rowsum row (no DRAM roundtrips)
  and per-head normalization overlaps the pair loop.
- DMA issue order prioritizes the critical path: consts, xq/xtq (LN chain),
  wkv + xt (kv chain, split in 4 column slices gating kv matmul chunks), wq,
  then bias tiles / wout.

Raw Block + explicit semaphores (this walrus build rejects multi-wait
instructions); the planner records semaphore counter targets at plan time,
then emits all four engine programs inside one Block.
"""

import sys
import numpy as np

sys.path.insert(0, "/opt/trn_rl_repo")

B, N, DIM, HEADS, DIM_HEAD, NUM_NULL = 1, 2048, 1024, 16, 64, 2
INNER = HEADS * DIM_HEAD
EPS = 1e-5
NCORES = 8
JT = 17
JPAD = JT * 128
NQ = 256
MASK_VAL = -30000.0
NPAIR = HEADS // 2

# per-head q-col start for key tile jt (worst case over cores => widest)
S_JT = [max(0, 16 * jt - 1) for jt in range(JT)]
W_JT = [256 - s for s in S_JT]          # per-head width
TW = [2 * w for w in W_JT]              # per-pair tile width (h0|h1 packed)
# E/bias pair layout: [h0 tiles packed (EW1) | h1 tiles packed (EW1)]
ECOL1 = [0] * JT
for _jt in range(1, JT):
    ECOL1[_jt] = ECOL1[_jt - 1] + W_JT[_jt - 1]
EW1 = ECOL1[-1] + W_JT[-1]              # 2192 per-head packed width
EW = 2 * EW1                            # 4384 packed pair E/bias width

# sim psum chunks: tiles packed into regions A(<=1536), B(<=1536), A(<=1536)
# region layout per chunk: [h0 tiles (L/2) | h1 tiles (L/2)]
CHUNKS = [[0, 1, 2], [3, 4, 5, 6], list(range(7, JT))]
CH_LEN = [sum(TW[j] for j in ck) for ck in CHUNKS]    # 1444, 1480, 1460
CH_E1 = [ECOL1[ck[0]] for ck in CHUNKS]               # per-head col offset
NCK = len(CHUNKS)

_CACHE = {}


def _build_graph():
    from contextlib import ExitStack
    import concourse.bass as bass
    import concourse.mybir as mybir

    dt = mybir.dt
    F32, BF16 = dt.float32, dt.bfloat16
    AF = mybir.ActivationFunctionType
    OP = mybir.AluOpType
    AX = mybir.AxisListType
    nc = bass.Bass()

    # all DRAM parameters are host-prearranged to match their SBUF layout
    # exactly (contiguous [128, W] rows -> 128 large DMA descriptors each);
    # xt is chunk-slice-major: [p, chs, ct, 512] so each of the 4 column
    # slices is one contiguous row-chunk
    # bcon packs bf16 consts+weights: ibf(0:128) ones(128:256) srow(256:1280)
    # nkvt(1280:1282) wkv(1282:2306); xf packs f32: xrow(0:2048)
    # xtq(2048:4096) if32(4096:4224) onesf(4224:4352)
    d_in = {}
    for name, shape, ty in [
        ("bcon", [128, 2306], BF16), ("xf", [128, 4352], F32),
        ("xt", [128, 8 * N], BF16), ("wq", [128, 8 * INNER], BF16),
        ("wout", [128, 8 * DIM], BF16), ("biasp", [NPAIR, 128, EW], BF16),
    ]:
        d_in[name] = nc.declare_dram_parameter(name, shape, ty, isOutput=False)
    out_d = nc.declare_dram_parameter("out", [DIM, NQ], F32, isOutput=True)

    ctx = ExitStack()
    sb = {}
    for name, shape, ty in [
        ("bcon", [128, 2306], BF16), ("xf", [128, 4352], F32),
        ("zb", [128, 1], F32), ("epsb", [128, 1], F32),
        ("wq", [128, 8 * INNER], BF16), ("wout", [128, 8 * DIM], BF16),
        ("xt", [128, 8 * N], BF16),
        ("xc", [128, DIM], F32),
        ("lns", [128, 12], F32),
        ("rsq_row", [1, NQ], F32), ("nmr_row", [1, NQ], F32),
        ("rsqb", [128, NQ], F32), ("negmurs", [1, NQ], BF16),
        ("xst", [128, 8 * NQ], BF16), ("qtmp", [128, 2 * NQ], BF16),
        ("kv", [128, JPAD], BF16), ("vsb", [128, JT * 65], BF16),
        ("e0", [128, EW], BF16), ("e1", [128, EW], BF16),
        ("b0", [128, EW], BF16), ("b1", [128, EW], BF16),
        ("eraw0", [128, CH_LEN[0]], BF16), ("eraw1", [128, CH_LEN[1]], BF16),
        ("eraw2", [128, CH_LEN[2]], BF16),
        ("oT", [64, HEADS * NQ], BF16),
        ("rrow", [1, 512], F32), ("recipflat", [1, HEADS * NQ], BF16),
        ("oTn", [128, 8 * NQ], BF16), ("oTn_lo", [64, 8 * NQ], BF16),
        ("outsb", [128, 3 * NQ], F32),
    ] + [(f"qh{h}", [64, 2 * NQ], BF16) for h in range(NPAIR)]:
        sb[name] = ctx.enter_context(nc.sbuf_tensor("sb_" + name, shape, ty))
    bc, xfm = sb["bcon"], sb["xf"]
    BC_IBF, BC_ONE, BC_SROW, BC_NKV, BC_WKV = 0, 128, 256, 1280, 1282
    XF_XR, XF_XTQ, XF_I32, XF_ONE = 0, 2048, 4096, 4224

    qh = [sb[f"qh{h}"] for h in range(NPAIR)]
    esb = [sb["e0"], sb["e1"]]
    bsb = [sb["b0"], sb["b1"]]
    eraw = [sb["eraw0"], sb["eraw1"], sb["eraw2"]]

    # PSUM: early tensors freed before pair-loop tensors are allocated.
    early = ExitStack()
    kvp = [early.enter_context(nc.psum_tensor(f"kvp{i}", [128, 512], F32))
           for i in range(2)]
    qp = [early.enter_context(nc.psum_tensor(f"qp{i}", [128, NQ], F32))
          for i in range(2)]
    vp = [early.enter_context(nc.psum_tensor(f"vp{i}", [128, 64], BF16))
          for i in range(2)]
    stp = early.enter_context(nc.psum_tensor("stp", [1, 128], F32))
    rbp = early.enter_context(nc.psum_tensor("rbp", [128, NQ], F32))
    early.close()
    simA = ctx.enter_context(nc.psum_tensor("simA", [128, 1536], F32))
    simB = ctx.enter_context(nc.psum_tensor("simB", [128, 1536], F32))
    opp = ctx.enter_context(nc.psum_tensor("opp", [65, 512], F32))
    nrm = ctx.enter_context(nc.psum_tensor("nrm", [64, 512], F32))
    SIMREG = [simA, simB]

    # ------- planner -------
    plan = {"sync": [], "tensor": [], "vector": [], "scalar": []}
    DSEMS = (("dbc", "dxf", "dwq", "dw", "db0", "db1", "do0", "do1", "do2")
             + tuple(f"dk{i}" for i in range(4))
             + tuple(f"dq{i}" for i in range(8))
             + tuple(f"dn{i}" for i in range(8)))
    cnt = {"p": 0, "v": 0, "s": 0, **{k: 0 for k in DSEMS}}
    SEM = {}

    def wait(eng, sem, thr):
        if thr > 0:
            plan[eng].append(lambda e, s=sem, t=thr: e.wait_ge(SEM[s], t))

    def dma(sem, out, in_):
        cnt[sem] += 16
        plan["sync"].append(
            lambda e, s=sem, o=out, i=in_: e.dma_start(out=o, in_=i)
            .then_inc(SEM[s], 16))
        return cnt[sem]

    def inc(eng, sem, fn):
        cnt[sem] += 1
        if eng in ("vector", "scalar"):
            plan[eng].append(lambda e, f=fn: f(e))
            plan[eng].append(lambda e, s=sem: e.drain().then_inc(SEM[s], 1))
        else:
            plan[eng].append(lambda e, f=fn, s=sem: f(e).then_inc(SEM[s], 1))
        return cnt[sem]

    def run(eng, fn):
        plan[eng].append(fn)
        if eng in ("vector", "scalar"):
            plan[eng].append(lambda e: e.drain())

    # ========== SYNC: initial loads in priority order ==========
    d_bc = dma("dbc", bc[:], d_in["bcon"][:])
    d_xf = dma("dxf", xfm[:], d_in["xf"][:])
    # xt split into 4 contiguous slices so kv matmul chunk ch gates on slice ch
    for chs in range(4):
        dma(f"dk{chs}", sb["xt"][:, chs * 4096:(chs + 1) * 4096],
            d_in["xt"][:, chs * 4096:(chs + 1) * 4096])
    d_wq = dma("dwq", sb["wq"][:], d_in["wq"][:])
    for p in range(2):
        dma(f"db{p}", bsb[p][:], d_in["biasp"][p])
    d_w = dma("dw", sb["wout"][:], d_in["wout"][:])

    # ========== VECTOR: memsets ==========
    run("vector", lambda e: e.memset(sb["zb"][:], 0.0))
    run("vector", lambda e: e.memset(sb["epsb"][:], EPS))
    run("vector", lambda e: e.memset(sb["vsb"][:], 1.0))
    run("vector", lambda e: e.memset(sb["kv"][:, NUM_NULL + N:JPAD], 0.0))
    wait("vector", "dbc", d_bc)
    v_memset = inc("vector", "v", lambda e: e.tensor_copy(
        sb["kv"][:, 0:NUM_NULL], bc[:, BC_NKV:BC_NKV + NUM_NULL]))

    # ========== LN stats: lns cols t*6 + {0 negmu, 1 ssq, 2 lnv, 3 rsqc, 4 nmrc}
    v_center = [0, 0]
    s_sq = [0, 0]
    s_rsqc = [0, 0]
    v_nmrc = [0, 0]
    for t in range(2):
        c0 = t * 6
        negmu = sb["lns"][:, c0:c0 + 1]
        if t == 0:
            wait("vector", "dxf", d_xf)
        if t == 1:
            wait("vector", "s", s_sq[0])  # xc reuse
        run("vector", lambda e, t=t, negmu=negmu: e.tensor_reduce(
            out=negmu, in_=xfm[:, t * DIM:(t + 1) * DIM],
            axis=AX.X, op=OP.add, negate=True))
        run("vector", lambda e, negmu=negmu: e.tensor_scalar_mul(
            out=negmu, in0=negmu, scalar1=1.0 / DIM))
        v_center[t] = inc("vector", "v", lambda e, t=t, negmu=negmu:
                          e.tensor_scalar_add(
                              out=sb["xc"][:],
                              in0=xfm[:, t * DIM:(t + 1) * DIM],
                              scalar1=negmu))
        # scalar chain for this t
        if t == 0:
            wait("scalar", "v", v_memset)
        wait("scalar", "v", v_center[t])
        ssq = sb["lns"][:, c0 + 1:c0 + 2]
        lnv = sb["lns"][:, c0 + 2:c0 + 3]
        rsqc = sb["lns"][:, c0 + 3:c0 + 4]
        s_sq[t] = inc("scalar", "s", lambda e, t=t, ssq=ssq: e.activation(
            out=xfm[:, t * DIM:(t + 1) * DIM], in_=sb["xc"][:],
            func=AF.Square, bias=sb["zb"][:], accum_out=ssq))
        run("scalar", lambda e, ssq=ssq, lnv=lnv: e.activation(
            out=lnv, in_=ssq, func=AF.Ln, scale=1.0 / DIM, bias=sb["epsb"][:]))
        s_rsqc[t] = inc("scalar", "s", lambda e, lnv=lnv, rsqc=rsqc: e.activation(
            out=rsqc, in_=lnv, func=AF.Exp, scale=-0.5, bias=sb["zb"][:]))
        wait("vector", "s", s_rsqc[t])
        v_nmrc[t] = inc("vector", "v", lambda e, c0=c0: e.tensor_tensor(
            out=sb["lns"][:, c0 + 4:c0 + 5], in0=sb["lns"][:, c0:c0 + 1],
            in1=sb["lns"][:, c0 + 3:c0 + 4], op=OP.mult))

    # ========== TENSOR: kv matmuls (kvp double-buffered) ==========
    p_kvchunk = [0] * 4
    s_kvevac = [0] * 4
    for ch in range(4):
        pb = kvp[ch % 2]
        if ch == 0:
            wait("tensor", "dbc", d_bc)
        wait("tensor", f"dk{ch}", 16)
        if ch >= 2:
            wait("tensor", "s", s_kvevac[ch - 2])
        for ct in range(8):
            fn = lambda e, pb=pb, ch=ch, ct=ct: e.matmul(
                pb[:], bc[:, BC_WKV + ct * 128:BC_WKV + (ct + 1) * 128],
                sb["xt"][:, ch * 4096 + ct * 512:ch * 4096 + (ct + 1) * 512],
                start=(ct == 0), stop=(ct == 7))
            if ct == 7:
                p_kvchunk[ch] = inc("tensor", "p", fn)
            else:
                run("tensor", fn)
        wait("scalar", "p", p_kvchunk[ch])
        s_kvevac[ch] = inc("scalar", "s", lambda e, pb=pb, ch=ch: e.activation(
            out=sb["kv"][:, NUM_NULL + ch * 512:NUM_NULL + (ch + 1) * 512],
            in_=pb[:], func=AF.Copy))

    # ========== TENSOR: stats transposes + rsqb broadcast ==========
    v_statrow = [[0, 0], [0, 0]]
    wait("tensor", "dxf", d_xf)   # if32/onesf loaded
    for t in range(2):
        c0 = t * 6
        wait("tensor", "s", s_rsqc[t])
        if t == 1:
            wait("tensor", "v", v_statrow[0][1])  # stp reuse
        pst = inc("tensor", "p", lambda e, c0=c0: e.transpose(
            stp[:], sb["lns"][:, c0 + 3:c0 + 4], xfm[:, XF_I32:XF_I32 + 128]))
        wait("vector", "p", pst)
        v_statrow[t][0] = inc("vector", "v", lambda e, t=t: e.tensor_copy(
            sb["rsq_row"][0:1, t * 128:(t + 1) * 128], stp[:]))
        wait("tensor", "v", v_statrow[t][0])
        wait("tensor", "v", v_nmrc[t])
        pst2 = inc("tensor", "p", lambda e, c0=c0: e.transpose(
            stp[:], sb["lns"][:, c0 + 4:c0 + 5], xfm[:, XF_I32:XF_I32 + 128]))
        wait("vector", "p", pst2)
        v_statrow[t][1] = inc("vector", "v", lambda e, t=t: e.tensor_copy(
            sb["nmr_row"][0:1, t * 128:(t + 1) * 128], stp[:]))

    wait("tensor", "v", v_statrow[1][0])
    p_rsqb = inc("tensor", "p", lambda e: e.matmul(
        rbp[:], xfm[0:1, XF_ONE:XF_ONE + 128], sb["rsq_row"][0:1, :],
        start=True, stop=True))
    wait("vector", "p", p_rsqb)
    run("vector", lambda e: e.tensor_copy(sb["rsqb"][:], rbp[:]))
    v_negmurs = inc("vector", "v",
                    lambda e: e.tensor_copy(sb["negmurs"][:], sb["nmr_row"][0:1, :]))
    for ct in range(8):
        fn = lambda e, ct=ct: e.tensor_tensor(
            out=sb["xst"][:, ct * NQ:(ct + 1) * NQ],
            in0=xfm[:, XF_XTQ + ct * NQ:XF_XTQ + (ct + 1) * NQ],
            in1=sb["rsqb"][:], op=OP.mult)
        if ct == 7:
            v_xst = inc("vector", "v", fn)
        else:
            run("vector", fn)

    # ========== TENSOR: v transposes (vp double-buffered) ==========
    p_vt = [0] * JT
    s_vcopy = [0] * JT
    for jt in range(JT):
        pb = vp[jt % 2]
        ch_hi = min(3, ((jt + 1) * 128 - 1 - NUM_NULL) // 512)
        wait("tensor", "s", s_kvevac[ch_hi])
        if jt == 0:
            wait("tensor", "v", v_memset)
        if jt >= 2:
            wait("tensor", "s", s_vcopy[jt - 2])
        p_vt[jt] = inc("tensor", "p", lambda e, pb=pb, jt=jt: e.transpose(
            pb[:], sb["kv"][64:128, jt * 128:(jt + 1) * 128],
            bc[64:128, BC_IBF + 64:BC_IBF + 128]))
        wait("scalar", "p", p_vt[jt])
        s_vcopy[jt] = inc("scalar", "s", lambda e, pb=pb, jt=jt: e.activation(
            out=sb["vsb"][:, jt * 65:jt * 65 + 64], in_=pb[:], func=AF.Copy))
    s_vsb = s_vcopy[JT - 1]

    # ========== TENSOR: q projection (qp double-buffered), head-major evac ===
    wait("tensor", "v", v_xst)
    wait("tensor", "dwq", d_wq)
    wait("tensor", "dbc", d_bc)
    p_q = [0] * 8
    v_qtmp = [0] * 8
    for dtile in range(8):
        pb = qp[dtile % 2]
        if dtile >= 2:
            wait("tensor", "v", v_qtmp[dtile - 2])
        for ct in range(8):
            run("tensor", lambda e, pb=pb, dtile=dtile, ct=ct: e.matmul(
                pb[:],
                sb["wq"][:, ct * INNER + dtile * 128:ct * INNER + (dtile + 1) * 128],
                sb["xst"][:, ct * NQ:(ct + 1) * NQ],
                start=(ct == 0), stop=False))
        p_q[dtile] = inc("tensor", "p", lambda e, pb=pb, dtile=dtile: e.matmul(
            pb[:], bc[0:1, BC_SROW + dtile * 128:BC_SROW + (dtile + 1) * 128],
            sb["negmurs"][:], start=False, stop=True))
        wait("vector", "p", p_q[dtile])
        # even head (psum rows 0:64) -> qh[p][:, 0:256] directly
        run("vector", lambda e, pb=pb, dtile=dtile: e.tensor_copy(
            qh[dtile][0:64, 0:NQ], pb[0:64, :]))
        slot = dtile % 2
        if dtile >= 2:
            wait("vector", f"dq{dtile - 2}", 16)  # qtmp slot reuse
        v_qtmp[dtile] = inc("vector", "v", lambda e, pb=pb, slot=slot:
                            e.tensor_copy(
                                sb["qtmp"][64:128, slot * NQ:(slot + 1) * NQ],
                                pb[64:128, :]))
        wait("sync", "v", v_qtmp[dtile])
        dma(f"dq{dtile}", qh[dtile][0:64, NQ:2 * NQ],
            sb["qtmp"][64:128, slot * NQ:(slot + 1) * NQ])

    # ========== PAIR LOOP ==========
    v_pre = v_qtmp[7]
    p_simc = [[0] * NCK for _ in range(NPAIR)]
    s_exp = [[0] * NCK for _ in range(NPAIR)]
    v_mult = [[0] * NCK for _ in range(NPAIR)]
    p_odone = [0] * NPAIR
    v_oevac = [0] * NPAIR
    p_bcast = [0] * NPAIR
    v_normmult = [0] * NPAIR

    def emit_fill(p, ci):
        # regions alternate by global chunk index: reuse guard is the exp of
        # the chunk two slots earlier, which finished two chunk-periods ago
        g = NCK * p + ci
        ps = SIMREG[g % 2]
        if g >= 2:
            pp, cp = divmod(g - 2, NCK)
            wait("tensor", "s", s_exp[pp][cp])
        if p == 0 and ci == 0:
            wait("tensor", "v", v_pre)      # early psum drained (qp/rbp/stp)
            wait("tensor", "s", s_vsb)      # vp drained + kvp via kvevacs
        if ci == 0:
            wait("tensor", f"dq{p}", 16)
        ch_hi = min(3, ((CHUNKS[ci][-1] + 1) * 128 - 1 - NUM_NULL) // 512)
        if p == 0:
            wait("tensor", "s", s_kvevac[ch_hi])
        base = CH_E1[ci]
        half = CH_LEN[ci] // 2
        # build emission list of bank-safe pieces, then set start on the
        # first piece touching each psum bank and stop on the last (start
        # zeroes the whole 2KB bank; one start/stop pair per bank per group)
        pieces = []
        for jt in CHUNKS[ci]:
            s, w = S_JT[jt], W_JT[jt]
            for h in range(2):
                a0 = h * half + (ECOL1[jt] - base)
                a, b = a0, a0 + w
                while a < b:
                    cut = min(b, (a // 512 + 1) * 512)
                    qa = h * 256 + s + (a - a0)
                    pieces.append([jt, a, cut, qa, qa + (cut - a)])
                    a = cut
        first_in_bank = {}
        last_in_bank = {}
        for pi, (jt, a, b, qa, qb) in enumerate(pieces):
            first_in_bank.setdefault(a // 512, pi)
            last_in_bank[a // 512] = pi
        for pi, (jt, a, b, qa, qb) in enumerate(pieces):
            st = first_in_bank[a // 512] == pi
            sp = last_in_bank[a // 512] == pi
            fn = lambda e, ps=ps, jt=jt, a=a, b=b, qa=qa, qb=qb, st=st, \
                sp=sp: e.matmul(
                    ps[:, a:b], sb["kv"][0:64, jt * 128:(jt + 1) * 128],
                    qh[p][0:64, qa:qb], start=st, stop=sp)
            if pi == len(pieces) - 1:
                p_simc[p][ci] = inc("tensor", "p", fn)
            else:
                run("tensor", fn)

    def emit_ogroup(p):
        eh_ = esb[p % 2]
        if p == 0:
            wait("tensor", "s", s_vsb)
        if p >= 1:
            wait("tensor", "v", v_oevac[p - 1])   # opp reuse
        for ci in range(NCK):
            wait("tensor", "v", v_mult[p][ci])
            for jt in CHUNKS[ci]:
                s, w = S_JT[jt], W_JT[jt]
                for h in range(2):
                    fn = lambda e, jt=jt, s=s, w=w, h=h, eh_=eh_: e.matmul(
                        opp[0:65, h * 256 + s:(h + 1) * 256],
                        sb["vsb"][:, jt * 65:jt * 65 + 65],
                        eh_[:, h * EW1 + ECOL1[jt]:h * EW1 + ECOL1[jt] + w],
                        start=(jt == 0 and h == 0),
                        stop=(jt == JT - 1 and h == 1))
                    if jt == JT - 1 and h == 1:
                        p_odone[p] = inc("tensor", "p", fn)
                    else:
                        run("tensor", fn)

    def emit_bcast(p):
        wait("tensor", "v", v_oevac[p])      # recipflat(p) ready
        if p >= 1:
            wait("tensor", "v", v_normmult[p - 1])   # nrm reuse
        p_bcast[p] = inc("tensor", "p", lambda e, p=p: e.matmul(
            nrm[:], bc[0:1, BC_ONE:BC_ONE + 64],
            sb["recipflat"][0:1, p * 512:(p + 1) * 512], start=True, stop=True))

    def emit_mults(p, ci):
        eh_ = esb[p % 2]
        bh_ = bsb[p % 2]
        wait("vector", "s", s_exp[p][ci])
        if ci == 0:
            wait("vector", f"db{p % 2}", 16 * (p // 2 + 1))
            if p >= 2:
                wait("vector", "p", p_odone[p - 2])   # eh slot reuse
        base = CH_E1[ci]
        half = CH_LEN[ci] // 2
        run("vector", lambda e, half=half, base=base, eh_=eh_, bh_=bh_,
            ci=ci: e.tensor_tensor(
                out=eh_[:, base:base + half], in0=eraw[ci][:, 0:half],
                in1=bh_[:, base:base + half], op=OP.mult))
        v_mult[p][ci] = inc("vector", "v", lambda e, half=half, base=base,
                            eh_=eh_, bh_=bh_, ci=ci: e.tensor_tensor(
            out=eh_[:, EW1 + base:EW1 + base + half],
            in0=eraw[ci][:, half:2 * half],
            in1=bh_[:, EW1 + base:EW1 + base + half], op=OP.mult))

    def emit_oevac(p):
        wait("vector", "p", p_odone[p])
        run("vector", lambda e, p=p: e.tensor_copy(
            sb["oT"][0:64, p * 512:(p + 1) * 512], opp[0:64, :]))
        run("vector", lambda e: e.reciprocal(
            out=sb["rrow"][0:1, :], in_=opp[64:65, :]))
        v_oevac[p] = inc("vector", "v", lambda e, p=p: e.tensor_copy(
            sb["recipflat"][0:1, p * 512:(p + 1) * 512], sb["rrow"][0:1, :]))

    def emit_normmult(p):
        wait("vector", "p", p_bcast[p])
        run("vector", lambda e, p=p: e.tensor_tensor(
            out=sb["oTn"][0:64, p * NQ:(p + 1) * NQ], in0=nrm[0:64, 0:256],
            in1=sb["oT"][0:64, p * 512:p * 512 + 256], op=OP.mult))
        v_normmult[p] = inc("vector", "v", lambda e, p=p: e.tensor_tensor(
            out=sb["oTn_lo"][0:64, p * NQ:(p + 1) * NQ], in0=nrm[0:64, 256:512],
            in1=sb["oT"][0:64, p * 512 + 256:(p + 1) * 512], op=OP.mult))

    def emit_exp(p, ci):
        wait("scalar", "p", p_simc[p][ci])
        if p >= 1:
            wait("scalar", "v", v_mult[p - 1][ci])    # eraw slot reuse
        ps = SIMREG[(NCK * p + ci) % 2]
        ln = CH_LEN[ci]
        s_exp[p][ci] = inc("scalar", "s", lambda e, ps=ps, ln=ln, ci=ci:
                           e.activation(out=eraw[ci][:, 0:ln], in_=ps[:, 0:ln],
                                        func=AF.Exp, bias=sb["zb"][:]))

    for p in range(NPAIR):
        # call order matters only for plan-time counter availability;
        # engine programs are built per-engine in the order emitted below
        emit_fill(p, 0)
        emit_fill(p, 1)
        emit_exp(p, 0)
        emit_exp(p, 1)
        if p >= 1:
            emit_ogroup(p - 1)
        if p >= 2:
            emit_bcast(p - 2)
        if p >= 1:
            emit_oevac(p - 1)
        if p >= 2:
            emit_normmult(p - 2)
        emit_mults(p, 0)
        emit_fill(p, 2)
        emit_exp(p, 2)
        emit_mults(p, 1)
        emit_mults(p, 2)
        # SYNC: bias prefetch for pair p+2; oTn odd-half shuffle for p-2
        if p + 2 < NPAIR:
            wait("sync", "v", v_mult[p][2])
            dma(f"db{(p + 2) % 2}", bsb[(p + 2) % 2][:], d_in["biasp"][p + 2])
        if p >= 2:
            wait("sync", "v", v_normmult[p - 2])
            dma(f"dn{p - 2}", sb["oTn"][64:128, (p - 2) * NQ:(p - 1) * NQ],
                sb["oTn_lo"][0:64, (p - 2) * NQ:(p - 1) * NQ])

    # epilogue
    emit_ogroup(NPAIR - 1)
    emit_bcast(NPAIR - 2)
    emit_oevac(NPAIR - 1)
    emit_normmult(NPAIR - 2)
    emit_bcast(NPAIR - 1)
    emit_normmult(NPAIR - 1)
    for p in (NPAIR - 2, NPAIR - 1):
        wait("sync", "v", v_normmult[p])
        dma(f"dn{p}", sb["oTn"][64:128, p * NQ:(p + 1) * NQ],
            sb["oTn_lo"][0:64, p * NQ:(p + 1) * NQ])

    # ========== output projection (accumulators alternate in simA) ==========
    # per head-half matmuls read oTn (even heads) / oTn_lo (odd) directly;
    # per-hdt waits let early heads accumulate while late pairs normalize
    wait("tensor", "dw", d_w)
    wait("tensor", "s", s_exp[NPAIR - 1][2])   # sim regions free
    s_outevac = [0] * 8
    OREGS = [simA[:, 0:NQ], simA[:, 512:512 + NQ], simA[:, 1024:1024 + NQ]]
    for et in range(8):
        reg = OREGS[et % 3]
        if et >= 3:
            wait("tensor", "s", s_outevac[et - 3])
        p_wout = 0
        for hdt in range(8):
            if et == 0:
                wait("tensor", f"dn{hdt}", 16)
            fn = lambda e, et=et, hdt=hdt, reg=reg: e.matmul(
                reg, sb["wout"][:, hdt * DIM + et * 128:hdt * DIM + (et + 1) * 128],
                sb["oTn"][:, hdt * NQ:(hdt + 1) * NQ],
                start=(hdt == 0), stop=(hdt == 7))
            if hdt == 7:
                p_wout = inc("tensor", "p", fn)
            else:
                run("tensor", fn)
        wait("scalar", "p", p_wout)
        oslot = et % 3
        if et >= 3:
            wait("scalar", f"do{oslot}", 16 * (et // 3))  # outsb slot reuse
        s_outevac[et] = inc("scalar", "s", lambda e, oslot=oslot, reg=reg:
                            e.activation(
                                out=sb["outsb"][:, oslot * NQ:(oslot + 1) * NQ],
                                in_=reg, func=AF.Copy))
        wait("sync", "s", s_outevac[et])
        dma(f"do{oslot}", out_d[et * 128:(et + 1) * 128, :],
            sb["outsb"][:, oslot * NQ:(oslot + 1) * NQ])

    # ========== emit ==========
    from contextlib import ExitStack as _ES
    semctx = _ES()
    for k in ("p", "v", "s") + DSEMS:
        SEM[k] = semctx.enter_context(nc.semaphore(f"sem_{k}"))
    with semctx:
        with nc.Block() as block:
            @block.sync
            def _(e):
                for fn in plan["sync"]:
                    fn(e)

            @block.tensor
            def _(e):
                for fn in plan["tensor"]:
                    fn(e)

            @block.vector
            def _(e):
                for fn in plan["vector"]:
                    fn(e)

            @block.scalar
            def _(e):
                for fn in plan["scalar"]:
                    fn(e)
    ctx.close()
    return nc


def _prep_inputs(x, attn_bias, Wq, Wkv, null_kv, Wout, gamma, mask):
    from ml_dtypes import bfloat16
    x = np.asarray(x, np.float32)[0]            # [N, DIM]
    attn_bias = np.asarray(attn_bias, np.float32)[0]  # [H, N, N]
    Wq = np.asarray(Wq, np.float32)
    Wkv = np.asarray(Wkv, np.float32)
    null_kv = np.asarray(null_kv, np.float32)
    Wout = np.asarray(Wout, np.float32)
    gamma = np.asarray(gamma, np.float32)
    mask = np.asarray(mask, bool)[0]            # [N]

    scale = DIM_HEAD ** -0.5
    wq_eff = (gamma[:, None] * Wq * scale).astype(np.float32)
    srow = wq_eff.sum(axis=0, keepdims=True)
    xt = np.ascontiguousarray(x.T)
    nkvt = np.zeros((128, NUM_NULL), np.float32)
    nkvt[0:DIM_HEAD, :] = null_kv[0].T
    nkvt[64:64 + DIM_HEAD, :] = null_kv[1].T
    I128 = np.eye(128, dtype=np.float32)
    ones = np.ones((1, 128), np.float32)

    jpad = np.arange(JPAD)
    jvalid = np.zeros(JPAD, bool)
    jvalid[:NUM_NULL] = True
    jvalid[NUM_NULL:NUM_NULL + N] = mask
    key_of_j = jpad - NUM_NULL

    in_maps = []
    idx_all = []
    for c in range(NCORES):
        idx = np.concatenate([np.arange(c, 1024, 8), np.arange(1024 + c, 2048, 8)])
        idx_all.append(idx)
        allow = jvalid[None, :] & (key_of_j[None, :] <= idx[:, None])  # [NQ, JPAD]
        allow[:, :NUM_NULL] = True
        ab = np.zeros((HEADS, JPAD, NQ), np.float32)
        ab[:, NUM_NULL:NUM_NULL + N, :] = attn_bias[:, idx, :].transpose(0, 2, 1)
        bt = np.where(allow.T[None], ab, MASK_VAL)
        ebt = np.exp(bt)                 # exp(bias); exactly 0 where masked
        # pack per head-PAIR, trimmed per-tile: [h0 (w) | h1 (w)] at ECOL[jt]
        pk = np.empty((NPAIR, 128, EW), np.float32)
        for jt in range(JT):
            s, w = S_JT[jt], W_JT[jt]
            c0 = ECOL1[jt]
            tile = ebt[:, jt * 128:(jt + 1) * 128, s:256]     # [H, 128, w]
            pk[:, :, c0:c0 + w] = tile[0::2]
            pk[:, :, EW1 + c0:EW1 + c0 + w] = tile[1::2]
        bcon = np.zeros((128, 2306), np.float32)
        bcon[:, 0:128] = I128
        bcon[0, 128:256] = 1.0
        bcon[0, 256:1280] = srow[0]
        bcon[:, 1280:1282] = nkvt
        bcon[:, 1282:2306] = Wkv.reshape(8, 128, 128).transpose(1, 0, 2) \
            .reshape(128, -1)
        xf = np.zeros((128, 4352), np.float32)
        xf[:, 0:2048] = x[idx].reshape(2, 128, DIM).transpose(1, 0, 2) \
            .reshape(128, -1)
        xf[:, 2048:4096] = xt[:, idx].reshape(8, 128, NQ).transpose(1, 0, 2) \
            .reshape(128, -1)
        xf[:, 4096:4224] = I128
        xf[0, 4224:4352] = 1.0
        in_maps.append({
            "bcon": bcon.astype(bfloat16),
            "xf": xf,
            "xt": np.ascontiguousarray(
                xt.reshape(8, 128, 4, 512).transpose(1, 2, 0, 3)
                .reshape(128, -1)).astype(bfloat16),
            "wq": np.ascontiguousarray(
                wq_eff.reshape(8, 128, INNER).transpose(1, 0, 2)
                .reshape(128, -1)).astype(bfloat16),
            "wout": np.ascontiguousarray(
                Wout.reshape(8, 128, DIM).transpose(1, 0, 2)
                .reshape(128, -1)).astype(bfloat16),
            "biasp": pk.astype(bfloat16),
        })
    return in_maps, idx_all


def _run(inputs, trace=False):
    from concourse.bass_utils import run_bass_kernel_spmd
    if "nc" not in _CACHE:
        _CACHE["nc"] = _build_graph()
    nc = _CACHE["nc"]
    in_maps, idx_all = _prep_inputs(**inputs)
    res = run_bass_kernel_spmd(nc, in_maps, list(range(NCORES)), trace=trace)
    out = np.zeros((B, N, DIM), np.float32)
    for c in range(NCORES):
        out[0, idx_all[c], :] = res.results[c]["out"].T
    return out, res


def kernel(**inputs):
    out, _ = _run(inputs, trace=False)
    return out


# revision 17
# speedup vs baseline: 1.3875x; 1.0604x over previous
"""Trainium2 8-core kernel for nn_Attention_70892730187933 (sparse multi-query attention).

Sharding: sequence-parallel over query rows. Core c owns rows {i : i % 8 == c},
as 2 blocks of 128 rows (block0 < 1024, block1 >= 1024). Key space padded to
17*128 = 2176 (incl. 2 null cols). No collectives; host concatenates rows.

v2 design vs baseline:
- Causal q-column trimming: for key tile jt, only q columns [S_jt, 256) per
  head can attend (S_jt = max(0, 16*jt-1), worst-case over cores); per-pair
  sim width drops 6656 -> 4384. q is packed head-major so each tile is one
  contiguous per-head slice.
- Bias is applied multiplicatively: host packs exp(bias) (0 where masked) and
  DVE multiplies it into exp(sim) at bf16 2x rate. This removes the
  identity-matmul bias adds (half of all sim PE work) entirely.
- Softmax normalization is inline: rowsums ride as a ones-column in V; DVE
  takes reciprocals straight from the PSUM rowsum row (no DRAM roundtrips)
  and per-head normalization overlaps the pair loop.
- DMA issue order prioritizes the critical path: consts, xq/xtq (LN chain),
  wkv + xt (kv chain, split in 4 column slices gating kv matmul chunks), wq,
  then bias tiles / wout.

Raw Block + explicit semaphores (this walrus build rejects multi-wait
instructions); the planner records semaphore counter targets at plan time,
then emits all four engine programs inside one Block.
"""

import sys
import numpy as np

sys.path.insert(0, "/opt/trn_rl_repo")

B, N, DIM, HEADS, DIM_HEAD, NUM_NULL = 1, 2048, 1024, 16, 64, 2
INNER = HEADS * DIM_HEAD
EPS = 1e-5
NCORES = 8
JT = 17
JPAD = JT * 128
NQ = 256
MASK_VAL = -30000.0
NPAIR = HEADS // 2

# per-head q-col start for key tile jt (worst case over cores => widest)
S_JT = [max(0, 16 * jt - 1) for jt in range(JT)]
W_JT = [256 - s for s in S_JT]          # per-head width
TW = [2 * w for w in W_JT]              # per-pair tile width (h0|h1 packed)
# E/bias pair layout: [h0 tiles packed (EW1) | h1 tiles packed (EW1)]
ECOL1 = [0] * JT
for _jt in range(1, JT):
    ECOL1[_jt] = ECOL1[_jt - 1] + W_JT[_jt - 1]
EW1 = ECOL1[-1] + W_JT[-1]              # 2192 per-head packed width
EW = 2 * EW1                            # 4384 packed pair E/bias width

# sim psum chunks: tiles packed into regions A(<=1536), B(<=1536), A(<=1536)
# region layout per chunk: [h0 tiles (L/2) | h1 tiles (L/2)]
CHUNKS = [[0, 1, 2], [3, 4, 5, 6], list(range(7, JT))]
CH_LEN = [sum(TW[j] for j in ck) for ck in CHUNKS]    # 1444, 1480, 1460
CH_E1 = [ECOL1[ck[0]] for ck in CHUNKS]               # per-head col offset
NCK = len(CHUNKS)

_CACHE = {}


def _build_graph():
    from contextlib import ExitStack
    import concourse.bass as bass
    import concourse.mybir as mybir

    dt = mybir.dt
    F32, BF16 = dt.float32, dt.bfloat16
    AF = mybir.ActivationFunctionType
    OP = mybir.AluOpType
    AX = mybir.AxisListType
    nc = bass.Bass()

    # all DRAM parameters are host-prearranged to match their SBUF layout
    # exactly (contiguous [128, W] rows -> 128 large DMA descriptors each);
    # xt is chunk-slice-major: [p, chs, ct, 512] so each of the 4 column
    # slices is one contiguous row-chunk
    # bcon packs bf16 consts+weights: ibf(0:128) ones(128:256) srow(256:1280)
    # nkvt(1280:1282) wkv(1282:2306); xf packs f32: xrow(0:2048)
    # xtq(2048:4096) if32(4096:4224) onesf(4224:4352)
    d_in = {}
    for name, shape, ty in [
        ("bcon", [128, 2306], BF16), ("xf", [128, 4352], F32),
        ("xt", [128, 8 * N], BF16), ("wq", [128, 8 * INNER], BF16),
        ("wout", [128, 8 * DIM], BF16), ("biasp", [NPAIR, 128, EW], BF16),
    ]:
        d_in[name] = nc.declare_dram_parameter(name, shape, ty, isOutput=False)
    out_d = nc.declare_dram_parameter("out", [DIM, NQ], F32, isOutput=True)

    ctx = ExitStack()
    sb = {}
    for name, shape, ty in [
        ("bcon", [128, 2306], BF16), ("xf", [128, 4352], F32),
        ("zb", [128, 1], F32), ("epsb", [128, 1], F32),
        ("wq", [128, 8 * INNER], BF16), ("wout", [128, 8 * DIM], BF16),
        ("xt", [128, 8 * N], BF16),
        ("xc", [128, DIM], F32),
        ("lns", [128, 12], F32),
        ("rsq_row", [1, NQ], F32), ("nmr_row", [1, NQ], F32),
        ("rsqb", [128, NQ], F32), ("negmurs", [1, NQ], BF16),
        ("xst", [128, 8 * NQ], BF16), ("qtmp", [128, 2 * NQ], BF16),
        ("kv", [128, JPAD], BF16), ("vsb", [128, JT * 65], BF16),
        ("e0", [128, EW], BF16), ("e1", [128, EW], BF16),
        ("b0", [128, EW], BF16), ("b1", [128, EW], BF16),
        ("eraw0", [128, CH_LEN[0]], BF16), ("eraw1", [128, CH_LEN[1]], BF16),
        ("eraw2", [128, CH_LEN[2]], BF16),
        ("oT", [64, HEADS * NQ], BF16),
        ("rrow", [1, 512], F32), ("recipflat", [1, HEADS * NQ], BF16),
        ("oTn", [128, 8 * NQ], BF16), ("oTn_lo", [64, 8 * NQ], BF16),
        ("outsb", [128, 3 * NQ], F32),
    ] + [(f"qh{h}", [64, 2 * NQ], BF16) for h in range(NPAIR)]:
        sb[name] = ctx.enter_context(nc.sbuf_tensor("sb_" + name, shape, ty))
    bc, xfm = sb["bcon"], sb["xf"]
    BC_IBF, BC_ONE, BC_SROW, BC_NKV, BC_WKV = 0, 128, 256, 1280, 1282
    XF_XR, XF_XTQ, XF_I32, XF_ONE = 0, 2048, 4096, 4224

    qh = [sb[f"qh{h}"] for h in range(NPAIR)]
    esb = [sb["e0"], sb["e1"]]
    bsb = [sb["b0"], sb["b1"]]
    eraw = [sb["eraw0"], sb["eraw1"], sb["eraw2"]]

    # PSUM: early tensors freed before pair-loop tensors are allocated.
    early = ExitStack()
    kvp = [early.enter_context(nc.psum_tensor(f"kvp{i}", [128, 512], F32))
           for i in range(2)]
    qp = [early.enter_context(nc.psum_tensor(f"qp{i}", [128, NQ], F32))
          for i in range(2)]
    vp = [early.enter_context(nc.psum_tensor(f"vp{i}", [128, 64], BF16))
          for i in range(2)]
    stp = early.enter_context(nc.psum_tensor("stp", [1, 128], F32))
    rbp = early.enter_context(nc.psum_tensor("rbp", [128, NQ], F32))
    early.close()
    simA = ctx.enter_context(nc.psum_tensor("simA", [128, 1536], F32))
    simB = ctx.enter_context(nc.psum_tensor("simB", [128, 1536], F32))
    opp = ctx.enter_context(nc.psum_tensor("opp", [65, 512], F32))
    nrm = ctx.enter_context(nc.psum_tensor("nrm", [64, 512], F32))
    SIMREG = [simA, simB]

    # ------- planner -------
    plan = {"sync": [], "tensor": [], "vector": [], "scalar": []}
    DSEMS = (("dbc", "dxf", "dwq", "dw", "db0", "db1", "do0", "do1", "do2")
             + tuple(f"dk{i}" for i in range(4))
             + tuple(f"dq{i}" for i in range(8))
             + tuple(f"dn{i}" for i in range(8)))
    cnt = {"p": 0, "v": 0, "s": 0, **{k: 0 for k in DSEMS}}
    SEM = {}

    def wait(eng, sem, thr):
        if thr > 0:
            plan[eng].append(lambda e, s=sem, t=thr: e.wait_ge(SEM[s], t))

    def dma(sem, out, in_):
        cnt[sem] += 16
        plan["sync"].append(
            lambda e, s=sem, o=out, i=in_: e.dma_start(out=o, in_=i)
            .then_inc(SEM[s], 16))
        return cnt[sem]

    def inc(eng, sem, fn):
        cnt[sem] += 1
        if eng in ("vector", "scalar"):
            plan[eng].append(lambda e, f=fn: f(e))
            plan[eng].append(lambda e, s=sem: e.drain().then_inc(SEM[s], 1))
        else:
            plan[eng].append(lambda e, f=fn, s=sem: f(e).then_inc(SEM[s], 1))
        return cnt[sem]

    def run(eng, fn):
        plan[eng].append(fn)
        if eng in ("vector", "scalar"):
            plan[eng].append(lambda e: e.drain())

    # ========== SYNC: initial loads in priority order ==========
    d_bc = dma("dbc", bc[:], d_in["bcon"][:])
    d_xf = dma("dxf", xfm[:], d_in["xf"][:])
    # xt split into 4 contiguous slices so kv matmul chunk ch gates on slice ch
    for chs in range(4):
        dma(f"dk{chs}", sb["xt"][:, chs * 4096:(chs + 1) * 4096],
            d_in["xt"][:, chs * 4096:(chs + 1) * 4096])
    d_wq = dma("dwq", sb["wq"][:], d_in["wq"][:])
    for p in range(2):
        dma(f"db{p}", bsb[p][:], d_in["biasp"][p])
    d_w = dma("dw", sb["wout"][:], d_in["wout"][:])

    # ========== VECTOR: memsets ==========
    run("vector", lambda e: e.memset(sb["zb"][:], 0.0))
    run("vector", lambda e: e.memset(sb["epsb"][:], EPS))
    run("vector", lambda e: e.memset(sb["vsb"][:], 1.0))
    run("vector", lambda e: e.memset(sb["kv"][:, NUM_NULL + N:JPAD], 0.0))
    wait("vector", "dbc", d_bc)
    v_memset = inc("vector", "v", lambda e: e.tensor_copy(
        sb["kv"][:, 0:NUM_NULL], bc[:, BC_NKV:BC_NKV + NUM_NULL]))

    # ========== LN stats: lns cols t*6 + {0 negmu, 1 ssq, 2 lnv, 3 rsqc, 4 nmrc}
    v_center = [0, 0]
    s_sq = [0, 0]
    s_rsqc = [0, 0]
    v_nmrc = [0, 0]
    for t in range(2):
        c0 = t * 6
        negmu = sb["lns"][:, c0:c0 + 1]
        if t == 0:
            wait("vector", "dxf", d_xf)
        if t == 1:
            wait("vector", "s", s_sq[0])  # xc reuse
        run("vector", lambda e, t=t, negmu=negmu: e.tensor_reduce(
            out=negmu, in_=xfm[:, t * DIM:(t + 1) * DIM],
            axis=AX.X, op=OP.add, negate=True))
        run("vector", lambda e, negmu=negmu: e.tensor_scalar_mul(
            out=negmu, in0=negmu, scalar1=1.0 / DIM))
        v_center[t] = inc("vector", "v", lambda e, t=t, negmu=negmu:
                          e.tensor_scalar_add(
                              out=sb["xc"][:],
                              in0=xfm[:, t * DIM:(t + 1) * DIM],
                              scalar1=negmu))
        # scalar chain for this t
        if t == 0:
            wait("scalar", "v", v_memset)
        wait("scalar", "v", v_center[t])
        ssq = sb["lns"][:, c0 + 1:c0 + 2]
        lnv = sb["lns"][:, c0 + 2:c0 + 3]
        rsqc = sb["lns"][:, c0 + 3:c0 + 4]
        s_sq[t] = inc("scalar", "s", lambda e, t=t, ssq=ssq: e.activation(
            out=xfm[:, t * DIM:(t + 1) * DIM], in_=sb["xc"][:],
            func=AF.Square, bias=sb["zb"][:], accum_out=ssq))
        run("scalar", lambda e, ssq=ssq, lnv=lnv: e.activation(
            out=lnv, in_=ssq, func=AF.Ln, scale=1.0 / DIM, bias=sb["epsb"][:]))
        s_rsqc[t] = inc("scalar", "s", lambda e, lnv=lnv, rsqc=rsqc: e.activation(
            out=rsqc, in_=lnv, func=AF.Exp, scale=-0.5, bias=sb["zb"][:]))
        wait("vector", "s", s_rsqc[t])
        v_nmrc[t] = inc("vector", "v", lambda e, c0=c0: e.tensor_tensor(
            out=sb["lns"][:, c0 + 4:c0 + 5], in0=sb["lns"][:, c0:c0 + 1],
            in1=sb["lns"][:, c0 + 3:c0 + 4], op=OP.mult))

    # ========== TENSOR: kv matmuls (kvp double-buffered) ==========
    p_kvchunk = [0] * 4
    s_kvevac = [0] * 4
    for ch in range(4):
        pb = kvp[ch % 2]
        if ch == 0:
            wait("tensor", "dbc", d_bc)
        wait("tensor", f"dk{ch}", 16)
        if ch >= 2:
            wait("tensor", "s", s_kvevac[ch - 2])
        for ct in range(8):
            fn = lambda e, pb=pb, ch=ch, ct=ct: e.matmul(
                pb[:], bc[:, BC_WKV + ct * 128:BC_WKV + (ct + 1) * 128],
                sb["xt"][:, ch * 4096 + ct * 512:ch * 4096 + (ct + 1) * 512],
                start=(ct == 0), stop=(ct == 7))
            if ct == 7:
                p_kvchunk[ch] = inc("tensor", "p", fn)
            else:
                run("tensor", fn)
        wait("scalar", "p", p_kvchunk[ch])
        s_kvevac[ch] = inc("scalar", "s", lambda e, pb=pb, ch=ch: e.activation(
            out=sb["kv"][:, NUM_NULL + ch * 512:NUM_NULL + (ch + 1) * 512],
            in_=pb[:], func=AF.Copy))

    # ========== TENSOR: stats transposes + rsqb broadcast ==========
    v_statrow = [[0, 0], [0, 0]]
    wait("tensor", "dxf", d_xf)   # if32/onesf loaded
    for t in range(2):
        c0 = t * 6
        wait("tensor", "s", s_rsqc[t])
        if t == 1:
            wait("tensor", "v", v_statrow[0][1])  # stp reuse
        pst = inc("tensor", "p", lambda e, c0=c0: e.transpose(
            stp[:], sb["lns"][:, c0 + 3:c0 + 4], xfm[:, XF_I32:XF_I32 + 128]))
        wait("vector", "p", pst)
        v_statrow[t][0] = inc("vector", "v", lambda e, t=t: e.tensor_copy(
            sb["rsq_row"][0:1, t * 128:(t + 1) * 128], stp[:]))
        wait("tensor", "v", v_statrow[t][0])
        wait("tensor", "v", v_nmrc[t])
        pst2 = inc("tensor", "p", lambda e, c0=c0: e.transpose(
            stp[:], sb["lns"][:, c0 + 4:c0 + 5], xfm[:, XF_I32:XF_I32 + 128]))
        wait("vector", "p", pst2)
        v_statrow[t][1] = inc("vector", "v", lambda e, t=t: e.tensor_copy(
            sb["nmr_row"][0:1, t * 128:(t + 1) * 128], stp[:]))

    wait("tensor", "v", v_statrow[1][0])
    p_rsqb = inc("tensor", "p", lambda e: e.matmul(
        rbp[:], xfm[0:1, XF_ONE:XF_ONE + 128], sb["rsq_row"][0:1, :],
        start=True, stop=True))
    wait("vector", "p", p_rsqb)
    run("vector", lambda e: e.tensor_copy(sb["rsqb"][:], rbp[:]))
    v_negmurs = inc("vector", "v",
                    lambda e: e.tensor_copy(sb["negmurs"][:], sb["nmr_row"][0:1, :]))
    for ct in range(8):
        fn = lambda e, ct=ct: e.tensor_tensor(
            out=sb["xst"][:, ct * NQ:(ct + 1) * NQ],
            in0=xfm[:, XF_XTQ + ct * NQ:XF_XTQ + (ct + 1) * NQ],
            in1=sb["rsqb"][:], op=OP.mult)
        if ct == 7:
            v_xst = inc("vector", "v", fn)
        else:
            run("vector", fn)

    # ========== TENSOR: v transposes (vp double-buffered) ==========
    p_vt = [0] * JT
    s_vcopy = [0] * JT
    for jt in range(JT):
        pb = vp[jt % 2]
        ch_hi = min(3, ((jt + 1) * 128 - 1 - NUM_NULL) // 512)
        wait("tensor", "s", s_kvevac[ch_hi])
        if jt == 0:
            wait("tensor", "v", v_memset)
        if jt >= 2:
            wait("tensor", "s", s_vcopy[jt - 2])
        p_vt[jt] = inc("tensor", "p", lambda e, pb=pb, jt=jt: e.transpose(
            pb[:], sb["kv"][64:128, jt * 128:(jt + 1) * 128],
            bc[64:128, BC_IBF + 64:BC_IBF + 128]))
        wait("scalar", "p", p_vt[jt])
        s_vcopy[jt] = inc("scalar", "s", lambda e, pb=pb, jt=jt: e.activation(
            out=sb["vsb"][:, jt * 65:jt * 65 + 64], in_=pb[:], func=AF.Copy))
    s_vsb = s_vcopy[JT - 1]

    # ========== TENSOR: q projection (qp double-buffered), head-major evac ===
    wait("tensor", "v", v_xst)
    wait("tensor", "dwq", d_wq)
    wait("tensor", "dbc", d_bc)
    p_q = [0] * 8
    v_qtmp = [0] * 8
    for dtile in range(8):
        pb = qp[dtile % 2]
        if dtile >= 2:
            wait("tensor", "v", v_qtmp[dtile - 2])
        for ct in range(8):
            run("tensor", lambda e, pb=pb, dtile=dtile, ct=ct: e.matmul(
                pb[:],
                sb["wq"][:, ct * INNER + dtile * 128:ct * INNER + (dtile + 1) * 128],
                sb["xst"][:, ct * NQ:(ct + 1) * NQ],
                start=(ct == 0), stop=False))
        p_q[dtile] = inc("tensor", "p", lambda e, pb=pb, dtile=dtile: e.matmul(
            pb[:], bc[0:1, BC_SROW + dtile * 128:BC_SROW + (dtile + 1) * 128],
            sb["negmurs"][:], start=False, stop=True))
        wait("vector", "p", p_q[dtile])
        # even head (psum rows 0:64) -> qh[p][:, 0:256] directly
        run("vector", lambda e, pb=pb, dtile=dtile: e.tensor_copy(
            qh[dtile][0:64, 0:NQ], pb[0:64, :]))
        slot = dtile % 2
        if dtile >= 2:
            wait("vector", f"dq{dtile - 2}", 16)  # qtmp slot reuse
        v_qtmp[dtile] = inc("vector", "v", lambda e, pb=pb, slot=slot:
                            e.tensor_copy(
                                sb["qtmp"][64:128, slot * NQ:(slot + 1) * NQ],
                                pb[64:128, :]))
        wait("sync", "v", v_qtmp[dtile])
        dma(f"dq{dtile}", qh[dtile][0:64, NQ:2 * NQ],
            sb["qtmp"][64:128, slot * NQ:(slot + 1) * NQ])

    # ========== PAIR LOOP ==========
    v_pre = v_qtmp[7]
    p_simc = [[0] * NCK for _ in range(NPAIR)]
    s_exp = [[0] * NCK for _ in range(NPAIR)]
    v_mult = [[0] * NCK for _ in range(NPAIR)]
    p_odone = [0] * NPAIR
    v_oevac = [0] * NPAIR
    p_bcast = [0] * NPAIR
    v_normmult = [0] * NPAIR

    def emit_fill(p, ci):
        # regions alternate by global chunk index: reuse guard is the exp of
        # the chunk two slots earlier, which finished two chunk-periods ago
        g = NCK * p + ci
        ps = SIMREG[g % 2]
        if g >= 2:
            pp, cp = divmod(g - 2, NCK)
            wait("tensor", "s", s_exp[pp][cp])
        if p == 0 and ci == 0:
            wait("tensor", "v", v_pre)      # early psum drained (qp/rbp/stp)
            wait("tensor", "s", s_vsb)      # vp drained + kvp via kvevacs
        if ci == 0:
            wait("tensor", f"dq{p}", 16)
        ch_hi = min(3, ((CHUNKS[ci][-1] + 1) * 128 - 1 - NUM_NULL) // 512)
        if p == 0:
            wait("tensor", "s", s_kvevac[ch_hi])
        base = CH_E1[ci]
        half = CH_LEN[ci] // 2
        # build emission list of bank-safe pieces, then set start on the
        # first piece touching each psum bank and stop on the last (start
        # zeroes the whole 2KB bank; one start/stop pair per bank per group)
        pieces = []
        for jt in CHUNKS[ci]:
            s, w = S_JT[jt], W_JT[jt]
            for h in range(2):
                a0 = h * half + (ECOL1[jt] - base)
                a, b = a0, a0 + w
                while a < b:
                    cut = min(b, (a // 512 + 1) * 512)
                    qa = h * 256 + s + (a - a0)
                    pieces.append([jt, a, cut, qa, qa + (cut - a)])
                    a = cut
        first_in_bank = {}
        last_in_bank = {}
        for pi, (jt, a, b, qa, qb) in enumerate(pieces):
            first_in_bank.setdefault(a // 512, pi)
            last_in_bank[a // 512] = pi
        for pi, (jt, a, b, qa, qb) in enumerate(pieces):
            st = first_in_bank[a // 512] == pi
            sp = last_in_bank[a // 512] == pi
            fn = lambda e, ps=ps, jt=jt, a=a, b=b, qa=qa, qb=qb, st=st, \
                sp=sp: e.matmul(
                    ps[:, a:b], sb["kv"][0:64, jt * 128:(jt + 1) * 128],
                    qh[p][0:64, qa:qb], start=st, stop=sp)
            if pi == len(pieces) - 1:
                p_simc[p][ci] = inc("tensor", "p", fn)
            else:
                run("tensor", fn)

    def emit_ogroup(p):
        eh_ = esb[p % 2]
        if p == 0:
            wait("tensor", "s", s_vsb)
        if p >= 1:
            wait("tensor", "v", v_oevac[p - 1])   # opp reuse
        for ci in range(NCK):
            wait("tensor", "v", v_mult[p][ci])
            for jt in CHUNKS[ci]:
                s, w = S_JT[jt], W_JT[jt]
                for h in range(2):
                    fn = lambda e, jt=jt, s=s, w=w, h=h, eh_=eh_: e.matmul(
                        opp[0:65, h * 256 + s:(h + 1) * 256],
                        sb["vsb"][:, jt * 65:jt * 65 + 65],
                        eh_[:, h * EW1 + ECOL1[jt]:h * EW1 + ECOL1[jt] + w],
                        start=(jt == 0 and h == 0),
                        stop=(jt == JT - 1 and h == 1))
                    if jt == JT - 1 and h == 1:
                        p_odone[p] = inc("tensor", "p", fn)
                    else:
                        run("tensor", fn)

    def emit_bcast(p):
        wait("tensor", "v", v_oevac[p])      # recipflat(p) ready
        if p >= 1:
            wait("tensor", "v", v_normmult[p - 1])   # nrm reuse
        p_bcast[p] = inc("tensor", "p", lambda e, p=p: e.matmul(
            nrm[:], bc[0:1, BC_ONE:BC_ONE + 64],
            sb["recipflat"][0:1, p * 512:(p + 1) * 512], start=True, stop=True))

    def emit_mults(p, ci):
        eh_ = esb[p % 2]
        bh_ = bsb[p % 2]
        wait("vector", "s", s_exp[p][ci])
        if ci == 0:
            wait("vector", f"db{p % 2}", 16 * (p // 2 + 1))
            if p >= 2:
                wait("vector", "p", p_odone[p - 2])   # eh slot reuse
        base = CH_E1[ci]
        half = CH_LEN[ci] // 2
        run("vector", lambda e, half=half, base=base, eh_=eh_, bh_=bh_,
            ci=ci: e.tensor_tensor(
                out=eh_[:, base:base + half], in0=eraw[ci][:, 0:half],
                in1=bh_[:, base:base + half], op=OP.mult))
        v_mult[p][ci] = inc("vector", "v", lambda e, half=half, base=base,
                            eh_=eh_, bh_=bh_, ci=ci: e.tensor_tensor(
            out=eh_[:, EW1 + base:EW1 + base + half],
            in0=eraw[ci][:, half:2 * half],
            in1=bh_[:, EW1 + base:EW1 + base + half], op=OP.mult))

    def emit_oevac(p):
        wait("vector", "p", p_odone[p])
        run("vector", lambda e, p=p: e.tensor_copy(
            sb["oT"][0:64, p * 512:(p + 1) * 512], opp[0:64, :]))
        run("vector", lambda e: e.reciprocal(
            out=sb["rrow"][0:1, :], in_=opp[64:65, :]))
        v_oevac[p] = inc("vector", "v", lambda e, p=p: e.tensor_copy(
            sb["recipflat"][0:1, p * 512:(p + 1) * 512], sb["rrow"][0:1, :]))

    def emit_normmult(p):
        wait("vector", "p", p_bcast[p])
        run("vector", lambda e, p=p: e.tensor_tensor(
            out=sb["oTn"][0:64, p * NQ:(p + 1) * NQ], in0=nrm[0:64, 0:256],
            in1=sb["oT"][0:64, p * 512:p * 512 + 256], op=OP.mult))
        v_normmult[p] = inc("vector", "v", lambda e, p=p: e.tensor_tensor(
            out=sb["oTn_lo"][0:64, p * NQ:(p + 1) * NQ], in0=nrm[0:64, 256:512],
            in1=sb["oT"][0:64, p * 512 + 256:(p + 1) * 512], op=OP.mult))

    def emit_exp(p, ci):
        wait("scalar", "p", p_simc[p][ci])
        if p >= 1:
            wait("scalar", "v", v_mult[p - 1][ci])    # eraw slot reuse
        ps = SIMREG[(NCK * p + ci) % 2]
        ln = CH_LEN[ci]
        s_exp[p][ci] = inc("scalar", "s", lambda e, ps=ps, ln=ln, ci=ci:
                           e.activation(out=eraw[ci][:, 0:ln], in_=ps[:, 0:ln],
                                        func=AF.Exp, bias=sb["zb"][:]))

    for p in range(NPAIR):
        # call order matters only for plan-time counter availability;
        # engine programs are built per-engine in the order emitted below
        emit_fill(p, 0)
        emit_fill(p, 1)
        emit_exp(p, 0)
        emit_exp(p, 1)
        emit_mults(p, 0)
        if p >= 1:
            emit_ogroup(p - 1)
        if p >= 2:
            emit_bcast(p - 2)
        emit_fill(p, 2)
        emit_exp(p, 2)
        emit_mults(p, 1)
        if p >= 1:
            emit_oevac(p - 1)
        if p >= 2:
            emit_normmult(p - 2)
        emit_mults(p, 2)
        # SYNC: bias prefetch for pair p+2; oTn odd-half shuffle for p-2
        if p + 2 < NPAIR:
            wait("sync", "v", v_mult[p][2])
            dma(f"db{(p + 2) % 2}", bsb[(p + 2) % 2][:], d_in["biasp"][p + 2])
        if p >= 2:
            wait("sync", "v", v_normmult[p - 2])
            dma(f"dn{p - 2}", sb["oTn"][64:128, (p - 2) * NQ:(p - 1) * NQ],
                sb["oTn_lo"][0:64, (p - 2) * NQ:(p - 1) * NQ])

    # epilogue
    emit_ogroup(NPAIR - 1)
    emit_bcast(NPAIR - 2)
    emit_oevac(NPAIR - 1)
    emit_normmult(NPAIR - 2)
    emit_bcast(NPAIR - 1)
    emit_normmult(NPAIR - 1)
    for p in (NPAIR - 2, NPAIR - 1):
        wait("sync", "v", v_normmult[p])
        dma(f"dn{p}", sb["oTn"][64:128, p * NQ:(p + 1) * NQ],
            sb["oTn_lo"][0:64, p * NQ:(p + 1) * NQ])

    # ========== output projection (accumulators alternate in simA) ==========
    # per head-half matmuls read oTn (even heads) / oTn_lo (odd) directly;
    # per-hdt waits let early heads accumulate while late pairs normalize
    wait("tensor", "dw", d_w)
    wait("tensor", "s", s_exp[NPAIR - 1][2])   # sim regions free
    s_outevac = [0] * 8
    OREGS = [simA[:, 0:NQ], simA[:, 512:512 + NQ], simA[:, 1024:1024 + NQ]]
    for et in range(8):
        reg = OREGS[et % 3]
        if et >= 3:
            wait("tensor", "s", s_outevac[et - 3])
        p_wout = 0
        for hdt in range(8):
            if et == 0:
                wait("tensor", f"dn{hdt}", 16)
            fn = lambda e, et=et, hdt=hdt, reg=reg: e.matmul(
                reg, sb["wout"][:, hdt * DIM + et * 128:hdt * DIM + (et + 1) * 128],
                sb["oTn"][:, hdt * NQ:(hdt + 1) * NQ],
                start=(hdt == 0), stop=(hdt == 7))
            if hdt == 7:
                p_wout = inc("tensor", "p", fn)
            else:
                run("tensor", fn)
        wait("scalar", "p", p_wout)
        oslot = et % 3
        if et >= 3:
            wait("scalar", f"do{oslot}", 16 * (et // 3))  # outsb slot reuse
        s_outevac[et] = inc("scalar", "s", lambda e, oslot=oslot, reg=reg:
                            e.activation(
                                out=sb["outsb"][:, oslot * NQ:(oslot + 1) * NQ],
                                in_=reg, func=AF.Copy))
        wait("sync", "s", s_outevac[et])
        dma(f"do{oslot}", out_d[et * 128:(et + 1) * 128, :],
            sb["outsb"][:, oslot * NQ:(oslot + 1) * NQ])

    # ========== emit ==========
    from contextlib import ExitStack as _ES
    semctx = _ES()
    for k in ("p", "v", "s") + DSEMS:
        SEM[k] = semctx.enter_context(nc.semaphore(f"sem_{k}"))
    with semctx:
        with nc.Block() as block:
            @block.sync
            def _(e):
                for fn in plan["sync"]:
                    fn(e)

            @block.tensor
            def _(e):
                for fn in plan["tensor"]:
                    fn(e)

            @block.vector
            def _(e):
                for fn in plan["vector"]:
                    fn(e)

            @block.scalar
            def _(e):
                for fn in plan["scalar"]:
                    fn(e)
    ctx.close()
    return nc


def _prep_inputs(x, attn_bias, Wq, Wkv, null_kv, Wout, gamma, mask):
    from ml_dtypes import bfloat16
    x = np.asarray(x, np.float32)[0]            # [N, DIM]
    attn_bias = np.asarray(attn_bias, np.float32)[0]  # [H, N, N]
    Wq = np.asarray(Wq, np.float32)
    Wkv = np.asarray(Wkv, np.float32)
    null_kv = np.asarray(null_kv, np.float32)
    Wout = np.asarray(Wout, np.float32)
    gamma = np.asarray(gamma, np.float32)
    mask = np.asarray(mask, bool)[0]            # [N]

    scale = DIM_HEAD ** -0.5
    wq_eff = (gamma[:, None] * Wq * scale).astype(np.float32)
    srow = wq_eff.sum(axis=0, keepdims=True)
    xt = np.ascontiguousarray(x.T)
    nkvt = np.zeros((128, NUM_NULL), np.float32)
    nkvt[0:DIM_HEAD, :] = null_kv[0].T
    nkvt[64:64 + DIM_HEAD, :] = null_kv[1].T
    I128 = np.eye(128, dtype=np.float32)
    ones = np.ones((1, 128), np.float32)

    jpad = np.arange(JPAD)
    jvalid = np.zeros(JPAD, bool)
    jvalid[:NUM_NULL] = True
    jvalid[NUM_NULL:NUM_NULL + N] = mask
    key_of_j = jpad - NUM_NULL

    in_maps = []
    idx_all = []
    for c in range(NCORES):
        idx = np.concatenate([np.arange(c, 1024, 8), np.arange(1024 + c, 2048, 8)])
        idx_all.append(idx)
        allow = jvalid[None, :] & (key_of_j[None, :] <= idx[:, None])  # [NQ, JPAD]
        allow[:, :NUM_NULL] = True
        ab = np.zeros((HEADS, JPAD, NQ), np.float32)
        ab[:, NUM_NULL:NUM_NULL + N, :] = attn_bias[:, idx, :].transpose(0, 2, 1)
        bt = np.where(allow.T[None], ab, MASK_VAL)
        ebt = np.exp(bt)                 # exp(bias); exactly 0 where masked
        # pack per head-PAIR, trimmed per-tile: [h0 (w) | h1 (w)] at ECOL[jt]
        pk = np.empty((NPAIR, 128, EW), np.float32)
        for jt in range(JT):
            s, w = S_JT[jt], W_JT[jt]
            c0 = ECOL1[jt]
            tile = ebt[:, jt * 128:(jt + 1) * 128, s:256]     # [H, 128, w]
            pk[:, :, c0:c0 + w] = tile[0::2]
            pk[:, :, EW1 + c0:EW1 + c0 + w] = tile[1::2]
        bcon = np.zeros((128, 2306), np.float32)
        bcon[:, 0:128] = I128
        bcon[0, 128:256] = 1.0
        bcon[0, 256:1280] = srow[0]
        bcon[:, 1280:1282] = nkvt
        bcon[:, 1282:2306] = Wkv.reshape(8, 128, 128).transpose(1, 0, 2) \
            .reshape(128, -1)
        xf = np.zeros((128, 4352), np.float32)
        xf[:, 0:2048] = x[idx].reshape(2, 128, DIM).transpose(1, 0, 2) \
            .reshape(128, -1)
        xf[:, 2048:4096] = xt[:, idx].reshape(8, 128, NQ).transpose(1, 0, 2) \
            .reshape(128, -1)
        xf[:, 4096:4224] = I128
        xf[0, 4224:4352] = 1.0
        in_maps.append({
            "bcon": bcon.astype(bfloat16),
            "xf": xf,
            "xt": np.ascontiguousarray(
                xt.reshape(8, 128, 4, 512).transpose(1, 2, 0, 3)
                .reshape(128, -1)).astype(bfloat16),
            "wq": np.ascontiguousarray(
                wq_eff.reshape(8, 128, INNER).transpose(1, 0, 2)
                .reshape(128, -1)).astype(bfloat16),
            "wout": np.ascontiguousarray(
                Wout.reshape(8, 128, DIM).transpose(1, 0, 2)
                .reshape(128, -1)).astype(bfloat16),
            "biasp": pk.astype(bfloat16),
        })
    return in_maps, idx_all


def _run(inputs, trace=False):
    from concourse.bass_utils import run_bass_kernel_spmd
    if "nc" not in _CACHE:
        _CACHE["nc"] = _build_graph()
    nc = _CACHE["nc"]
    in_maps, idx_all = _prep_inputs(**inputs)
    res = run_bass_kernel_spmd(nc, in_maps, list(range(NCORES)), trace=trace)
    out = np.zeros((B, N, DIM), np.float32)
    for c in range(NCORES):
        out[0, idx_all[c], :] = res.results[c]["out"].T
    return out, res


def kernel(**inputs):
    out, _ = _run(inputs, trace=False)
    return out
